# revision 9
# baseline (speedup 1.0000x reference)
"""
Trainium2 Bass kernel for nn_ABSA_Lstm: masked LSTM over ragged sequences.

  reference:  x = emb[sent]; LSTM over T=128 steps with per-sequence length
              masking; out = h_final @ Wout.T + bout   -> [256, 3]

Strategy (8 NeuronCores, data parallel):
  - Shard batch B=256 -> 32 sequences per core. LSTM weights replicated.
  - Host does the embedding-table gather (pure data movement) and packs
    transposed/padded tile layouts; all model FLOPs run on device:
      phase 1: gates_x[b,t,:] = x[b,t,:] @ Wih_r.T + (b_ih+b_hh)   (big matmul)
      phase 2: 128 sequential LSTM cell steps (h.T is the matmul stationary)
      phase 3: out = h_cap @ Wout.T + bout
  - Ragged lengths: the recurrence runs unmasked; h is *captured* into Hf at
    t == len[b]-1 via a per-partition one-hot scalar (off the critical path).
    Exact: for t >= len the reference state is frozen.

Performance structure (per recurrence step, Tile-scheduled):
  - ALL 4 gate chunks land in ONE PSUM bank tile pg[128, 300] with partitions
    = 4 gate-chunks x 32 batch: each Whh matmul routes its M=32 output to
    partition block 32j via the PE column-tile (tile_position inferred from
    out.base_partition()). Elementwise cost on TRN2 engines is
    free-size-driven, so the [128, 300] layout is ~4x cheaper than [32,1200].
  - ONE sigmoid covers all gates: host pre-scales the g-gate rows of
    Wih/Whh/bias by 2, so pg holds 2*gtilde there, and
    tanh(g) = 2*sigmoid(2g) - 1 is folded into the downstream
    scalar_tensor_tensor ops at zero extra cost:
        ig2 = (sig_g - 0.5) * sig_i          (= i * tanh(g) / 2)
        c   = ig2 * 2 + f*c                  (scalar_tensor_tensor)
  - gates_x is injected into PSUM by K=32 identity-stationary matmuls at
    row-group t%4 / col-group 32j (start=True), no dependency on h;
  - capture is one op: Hf += h * mlast[:, t] (mlast one-hot over t);
  - all elementwise traffic is bf16 (DVE 2x mode); engine balance: sigmoid +
    tanh(c) on Act, stt folds + capture + hT copies on DVE, f*c and o*tanh(c)
    on GPSIMD;
  - the Wih "phase 1" M-tiles are interleaved into the loop as PE filler
    bursts LAG tiles ahead of consumption, hiding the input projection
    behind the recurrence and keeping the PE pstate warm. Gate order is
    host-permuted to (g,i,f,o); weight/bias rows carry a ones-row so biases
    cost zero instructions.

This walrus build accepts only ONE sync wait per instruction, so after Tile
scheduling, extra waits are hoisted onto standalone EventSemaphore
instructions (_legalize_single_wait).
"""

import numpy as np
import ml_dtypes

import concourse.bass as bass
import concourse.tile as tile
from concourse import mybir
from concourse.bass_utils import run_bass_kernel_spmd

BF16 = ml_dtypes.bfloat16

# Model dims (hardcoded per spec nn_ABSA_Lstm_377957122440)
VOCAB, TVOCAB, D, H, C, B, T = 100000, 2000, 300, 300, 3, 256, 128
NCORES = 8
BL = B // NCORES          # 32 local batch
KT = 3                    # K tiles of 128 covering D(+1 bias row) / H
TC = T // 4               # 32 M-tiles of (4 t's x 32 b) in phase 1
NCH = 4                   # 1200 gate dims as 4 gate-major chunks of 300
CH = 300

_cache = {}


def _build_graph(legalize=True, debug=False, t_steps=T, reps=1,
                 trace_sim=False, variant=4):
    nc = bass.Bass()
    f32 = mybir.dt.float32
    bf16 = mybir.dt.bfloat16

    # ---- DRAM I/O ----
    xT = nc.dram_tensor("xT", [128, TC, KT, 128], bf16, kind="ExternalInput")
    wihT = nc.dram_tensor("wihT", [128, KT, 1200], bf16, kind="ExternalInput")
    whhT = nc.dram_tensor("whhT", [128, KT, 1200], bf16, kind="ExternalInput")
    wot = nc.dram_tensor("wot", [128, KT, 8], bf16, kind="ExternalInput")
    boutb = nc.dram_tensor("boutb", [BL, C], f32, kind="ExternalInput")
    h0T = nc.dram_tensor("h0T", [128, KT, BL], bf16, kind="ExternalInput")
    c0 = nc.dram_tensor("c0", [BL, H], bf16, kind="ExternalInput")
    mlast = nc.dram_tensor("mlast", [BL, T], bf16, kind="ExternalInput")
    ident = nc.dram_tensor("ident", [128, 32], bf16, kind="ExternalInput")
    out = nc.dram_tensor("out", [BL, C], f32, kind="ExternalOutput")
    dbg = None
    if debug:
        dbg = {
            "dbg_gates": nc.dram_tensor("dbg_gates", [128, CH], bf16,
                                        kind="ExternalOutput"),
            "dbg_h": nc.dram_tensor("dbg_h", [BL, H], bf16,
                                    kind="ExternalOutput"),
        }

    with tile.TileContext(nc, trace_sim=trace_sim) as tc:
        for _ in range(reps):
            _body(nc, tc, xT, wihT, whhT, wot, boutb, h0T, c0, mlast, ident,
                  out, dbg, t_steps, variant)
    if legalize:
        _legalize_single_wait(nc)
    return nc


def _legalize_single_wait(nc):
    """This walrus build accepts at most ONE sync wait per instruction.
    Hoist extra waits emitted by Tile onto standalone EventSemaphore
    instructions placed immediately before the offender on the same engine."""
    for fn in nc.m.functions:
        for b in fn.blocks:
            out = []
            for inst in b.instructions:
                si = getattr(inst, "sync_info", None)
                if si is not None and si.on_wait and len(si.on_wait) > 1:
                    for w in si.on_wait[:-1]:
                        out.append(mybir.InstEventSemaphore(
                            name=nc.get_next_instruction_name(),
                            engine=inst.engine,
                            ins=[], outs=[],
                            sync_info=mybir.SyncInfo(on_wait=[w], on_update=[]),
                        ))
                    si.on_wait = [si.on_wait[-1]]
                out.append(inst)
            b.instructions[:] = out


def _body(nc, tc, xT, wihT, whhT, wot, boutb, h0T, c0, mlast, ident, out,
          dbg=None, t_steps=T, variant=4):
    f32 = mybir.dt.float32
    bf16 = mybir.dt.bfloat16
    Sig = mybir.ActivationFunctionType.Sigmoid
    Tanh = mybir.ActivationFunctionType.Tanh
    MUL = mybir.AluOpType.mult
    ADD = mybir.AluOpType.add
    SUB = mybir.AluOpType.subtract

    from contextlib import ExitStack

    with ExitStack() as ctx:
        singles = ctx.enter_context(tc.tile_pool(name="singles", bufs=1))

        # ---- resident tiles ----
        whh_sb = singles.tile([128, KT, 1200], bf16)
        nc.sync.dma_start(out=whh_sb, in_=whhT[:, :, :])
        mlast_sb = singles.tile([BL, T], bf16)
        nc.sync.dma_start(out=mlast_sb, in_=mlast[:, :])
        ident_sb = singles.tile([128, 32], bf16)
        nc.sync.dma_start(out=ident_sb, in_=ident[:, :])
        boutb_sb = singles.tile([BL, C], f32)
        nc.sync.dma_start(out=boutb_sb, in_=boutb[:, :])

        gx_all = singles.tile([128, TC, 1200], bf16)
        xT_sb = singles.tile([128, TC, KT, 128], bf16)
        nc.sync.dma_start(out=xT_sb, in_=xT[:, :, :, :])

        # c lives at partition base 32 (paired with the f-gate rows of sig);
        # tanh(c) lives at base 96 (paired with the o-gate rows).
        c_al = singles.tile([64, H], bf16)
        nc.gpsimd.dma_start(out=c_al[32:64, :], in_=c0[:, :])
        Hf = singles.tile([BL, H], bf16)
        nc.vector.memset(Hf, 0.0)
        # ping-pong transposed hidden state [kt, 128, 32]
        hT = [singles.tile([128, KT, BL], bf16, name=f"hT{i}", tag=f"hT{i}")
              for i in range(2)]
        nc.vector.memset(hT[1], 0.0)
        nc.sync.dma_start(out=hT[0], in_=h0T[:, :, :])

        # ===== phases 1+2 fused: gates_x tiles computed inside the loop =====
        LAG = 2
        wih_sb = singles.tile([128, KT, 1200], bf16)
        nc.sync.dma_start(out=wih_sb, in_=wihT[:, :, :])

        with ExitStack() as p2:
            pgpool = p2.enter_context(tc.tile_pool(name="pgp", bufs=2, space="PSUM"))
            ptpool = p2.enter_context(tc.tile_pool(name="ptp", bufs=2, space="PSUM"))
            pspool = p2.enter_context(tc.tile_pool(name="psp", bufs=2, space="PSUM"))
            ew = p2.enter_context(tc.tile_pool(name="ew", bufs=2))

            def p1_burst(tcix, half):
                # compute gx_all[:, tcix, half*600 : half*600+600]
                ps = pspool.tile([128, 2, 512], f32, name=f"ps{tcix}_{half}",
                                 tag="ps")
                for k in range(KT):
                    for jj in range(2):
                        j = 2 * half + jj
                        nc.tensor.matmul(
                            ps[:, jj, 0:CH],
                            lhsT=xT_sb[:, tcix, k, :],
                            rhs=wih_sb[:, k, j * CH:(j + 1) * CH],
                            start=(k == 0), stop=(k == KT - 1),
                        )
                for jj in range(2):
                    j = 2 * half + jj
                    nc.scalar.activation(
                        gx_all[:, tcix, j * CH:(j + 1) * CH], ps[:, jj, 0:CH],
                        mybir.ActivationFunctionType.Copy)

            nprod = (t_steps + 3) // 4 if t_steps else 0
            for tcix in range(min(LAG, nprod)):
                p1_burst(tcix, 0)
                p1_burst(tcix, 1)

            for t in range(t_steps):
                tcix, tt = t // 4, t % 4
                cur, nxt = hT[t % 2], hT[(t + 1) % 2]

                # one PSUM bank holds all 4 gate chunks:
                # pg[32j + b, n] = gates[b, j*300 + n]  (gate order g,i,f,o;
                # g rows hold 2*gtilde via host-doubled weights).
                pg = pgpool.tile([128, 512], f32, name=f"pg{t}", tag="pg")
                # gx inject: no dependency on h -> overlaps previous step's
                # elementwise chain.
                for j in range(NCH):
                    nc.tensor.matmul(
                        pg[32 * j:32 * j + 32, 0:CH],
                        lhsT=ident_sb[32 * tt:32 * tt + 32, :],
                        rhs=gx_all[32 * tt:32 * tt + 32, tcix,
                                   j * CH:(j + 1) * CH],
                        start=True, stop=False,
                        tile_position=(32 * tt, 32 * j),
                    )
                for j in range(NCH):
                    for k in range(KT):
                        nc.tensor.matmul(
                            pg[32 * j:32 * j + 32, 0:CH],
                            lhsT=cur[:, k, :],
                            rhs=whh_sb[:, k, j * CH:(j + 1) * CH],
                            start=False, stop=(k == KT - 1),
                            tile_position=(0, 32 * j),
                        )

                if variant < 2:
                    continue

                # ONE sigmoid over all 4 gate chunks (bf16 out).
                # sig partition rows: g@0, f@32, i@64, o@96.
                sig = ew.tile([128, CH], bf16)
                nc.scalar.activation(sig, pg[:, 0:CH], Sig)
                if dbg is not None and t == 0:
                    nc.sync.dma_start(out=dbg["dbg_gates"][:, :], in_=sig)

                if variant < 3:
                    continue
                # SBUF input PAIRS must share a start partition (walrus rule),
                # so realign i-rows to base 0; c sits at base 32 (pairs with
                # f), tanh(c) at base 96 (pairs with o). Outputs may sit at
                # any base.
                lo_i = ew.tile([BL, H], bf16)
                nc.gpsimd.tensor_copy(lo_i, sig[64:96, :])
                fc = ew.tile([BL, H], bf16)
                nc.gpsimd.tensor_mul(fc, sig[32:64, :], c_al[32:64, :])
                # c = f*c + i*tanh(g);  i*tanh(g) = 2*(sig_g - 0.5)*sig_i
                ig2 = ew.tile([BL, H], bf16)
                nc.vector.scalar_tensor_tensor(
                    ig2, sig[0:32, :], 0.5, lo_i, SUB, MUL)
                nc.vector.scalar_tensor_tensor(
                    c_al[32:64, :], ig2, 2.0, fc, MUL, ADD)

                # h = o * tanh(c)
                th = ew.tile([128, CH], bf16, name=f"th{t}", tag="th")
                nc.scalar.activation(th[96:128, :], c_al[32:64, :], Tanh)
                h_bf = ew.tile([BL, H], bf16)
                nc.gpsimd.tensor_mul(h_bf, sig[96:128, :], th[96:128, :])

                if dbg is not None and t == 0:
                    nc.sync.dma_start(out=dbg["dbg_h"][:, :], in_=h_bf)

                # capture h at t == len-1 (one-hot mask): Hf += h * m_t
                nc.vector.scalar_tensor_tensor(
                    Hf, h_bf, mlast_sb[:, t:t + 1], Hf, MUL, ADD)

                if variant < 4:
                    continue
                # transpose h for next step's stationary
                pt = ptpool.tile([128, KT, 32], bf16, name=f"pt{t}", tag="pt")
                for k in range(KT):
                    w = 128 if k < 2 else H - 256
                    nc.tensor.transpose(
                        pt[0:w, k, :], h_bf[:, 128 * k:128 * k + w],
                        ident_sb[0:32, :])
                    nc.vector.tensor_copy(nxt[0:w, k, :], pt[0:w, k, :])

                # phase-1 filler burst for a future tile (keeps PE warm)
                if tt == 0 and tcix + LAG < nprod:
                    p1_burst(tcix + LAG, 0)
                elif tt == 2 and tcix + LAG < nprod:
                    p1_burst(tcix + LAG, 1)

        # ================= phase 3: out = Hf @ WoutT + bout =================
        with ExitStack() as p3:
            fp = p3.enter_context(tc.tile_pool(name="fp", bufs=1))
            fps = p3.enter_context(tc.tile_pool(name="fps", bufs=4, space="PSUM"))
            wot_sb = fp.tile([128, KT, 8], bf16)
            nc.sync.dma_start(out=wot_sb, in_=wot[:, :, :])
            hfT = fp.tile([128, KT, BL], bf16)
            nc.vector.memset(hfT, 0.0)
            for k in range(KT):
                w = 128 if k < 2 else H - 256
                pt = fps.tile([128, BL], bf16)
                nc.tensor.transpose(pt[0:w, :], Hf[:, 128 * k:128 * k + w],
                                    ident_sb[0:32, :])
                nc.vector.tensor_copy(hfT[0:w, k, :], pt[0:w, :])
            po = fps.tile([BL, 8], f32)
            for k in range(KT):
                nc.tensor.matmul(po[:, 0:C], lhsT=hfT[:, k, :], rhs=wot_sb[:, k, 0:C],
                                 start=(k == 0), stop=(k == KT - 1))
            o_sb = fp.tile([BL, C], f32)
            nc.vector.tensor_add(o_sb, po[:, 0:C], boutb_sb)
            nc.sync.dma_start(out=out[:, :], in_=o_sb)


def _prep_inputs(sent, target, lens, emb, Wih, Whh, b_ih, b_hh, h0, c0,
                 Wout, bout):
    """Host-side shard + layout packing (data movement / tiny reindexing only)."""
    # permute gate order (i,f,g,o) -> (g,f,i,o); double g rows for the
    # tanh(g) = 2*sigmoid(2g) - 1 trick.
    perm = np.concatenate([np.arange(600, 900), np.arange(300, 600),
                           np.arange(0, 300), np.arange(900, 1200)])
    wih_r = Wih[perm].astype(np.float32)          # [1200, 300]
    whh_r = Whh[perm].astype(np.float32)
    bias_r = (b_ih + b_hh)[perm].astype(np.float32)
    wih_r[0:300] *= 2.0
    whh_r[0:300] *= 2.0
    bias_r[0:300] *= 2.0

    # [p, kt, n] with row D==bias, zero padded
    wihT = np.zeros((128, KT, 1200), np.float32)
    whhT = np.zeros((128, KT, 1200), np.float32)
    for k in range(KT):
        lo, hi = 128 * k, min(128 * (k + 1), D)
        wihT[0:hi - lo, k, :] = wih_r[:, lo:hi].T
        whhT[0:hi - lo, k, :] = whh_r[:, lo:hi].T
    wihT[D - 256, 2, :] = bias_r                   # ones-row partner
    wot = np.zeros((128, KT, 8), np.float32)
    for k in range(KT):
        lo, hi = 128 * k, min(128 * (k + 1), H)
        wot[0:hi - lo, k, 0:C] = Wout[:, lo:hi].T

    ident = np.tile(np.eye(32, dtype=np.float32), (4, 1))

    in_maps = []
    for ci in range(NCORES):
        sl = slice(ci * BL, (ci + 1) * BL)
        x = emb[sent[sl]].astype(np.float32)       # [32, 128, 300] gather
        xT = np.zeros((128, TC, KT, 128), np.float32)
        # lhsT layout: xT[p=dk, tc, kt, m=32tt+b] = x[b, 4tc+tt, 128kt+dk]
        xr = x.transpose(1, 0, 2).reshape(TC, 4, BL, D)   # [tc, tt, b, d]
        xr = xr.reshape(TC, 128, D)                       # [tc, m, d]
        for k in range(KT):
            lo, hi = 128 * k, min(128 * (k + 1), D)
            xT[0:hi - lo, :, k, :] = xr[:, :, lo:hi].transpose(2, 0, 1)
        xT[D - 256, :, 2, :] = 1.0                 # bias ones-row

        h0T = np.zeros((128, KT, BL), np.float32)
        for k in range(KT):
            lo, hi = 128 * k, min(128 * (k + 1), H)
            h0T[0:hi - lo, k, :] = h0[sl, lo:hi].T

        lloc = lens[sl].astype(np.int64)
        mlast = np.zeros((BL, T), np.float32)
        mlast[np.arange(BL), np.clip(lloc - 1, 0, T - 1)] = 1.0

        in_maps.append({
            "xT": xT.astype(BF16),
            "wihT": wihT.astype(BF16),
            "whhT": whhT.astype(BF16),
            "wot": wot.astype(BF16),
            "boutb": np.tile(bout.astype(np.float32), (BL, 1)),
            "h0T": h0T.astype(BF16),
            "c0": c0[sl].astype(BF16),
            "mlast": mlast.astype(BF16),
            "ident": ident.astype(BF16),
        })
    return in_maps


def kernel(**inputs):
    if "nc" not in _cache:
        _cache["nc"] = _build_graph()
    nc = _cache["nc"]
    in_maps = _prep_inputs(**inputs)
    res = run_bass_kernel_spmd(nc, in_maps, core_ids=list(range(NCORES)))
    outs = [res.results[i]["out"] for i in range(NCORES)]
    return np.concatenate(outs, axis=0).astype(np.float32)


# revision 13
# speedup vs baseline: 2.1563x; 2.1563x over previous
"""
Trainium2 Bass kernel for nn_ABSA_Lstm: masked LSTM over ragged sequences.

  reference:  x = emb[sent]; LSTM over T=128 steps with per-sequence length
              masking; out = h_final @ Wout.T + bout   -> [256, 3]

Strategy (8 NeuronCores, data parallel):
  - Shard batch B=256 -> 32 sequences per core. LSTM weights replicated.
  - Host does the embedding-table gather (pure data movement) and packs
    transposed/padded tile layouts; all model FLOPs run on device.

FULLY TRANSPOSED formulation: every on-chip tensor carries feature dims on
PARTITIONS and batch on the FREE dim.  Engine cost on TRN2 is free-size
driven, so with batch (32) as the free dim every instruction is near its
fixed-cost floor, and:
  - gates^T [1200, 32] accumulate in ONE PSUM bank as [128p, 12 chunks, 32]
    (per-gate hd-chunks of 128/128/44 rows; gate order g,f,i,o; stationary
    weight columns are zero-padded to 128 so no partition garbage exists);
  - the input projection runs DIRECTLY into each step's PSUM bank
    (start=True) from resident xT tiles -- no gx buffer, no inject matmuls,
    no PSUM->SBUF drain;
  - h^T is produced by the elementwise chain already transposed -- the
    per-step PE transpose + copy of the h-major formulation disappears;
  - ONE sigmoid covers all 4 gates (host pre-doubles the g-gate rows;
    tanh(g) = 2*sigmoid(2g)-1 via a single 2x-mode tensor_scalar);
  - all elementwise ops are 2x-capable TensorTensor/TensorScalar at
    partition base 0 (no start-partition pairing issues);
  - ragged lengths: recurrence runs unmasked, Hf^T += h^T * mrep[:, :, t, :]
    with a one-hot-in-t replicated mask (two Pool ops, off critical path).
  - bias rides a ones-row in the xT k=2 tile (row 44) against a bias row in
    the Wih stationary; biases cost zero instructions.

This walrus build accepts only ONE sync wait per instruction, so after Tile
scheduling, extra waits are hoisted onto standalone EventSemaphore
instructions (_legalize_single_wait).
"""

import numpy as np
import ml_dtypes

import concourse.bass as bass
import concourse.tile as tile
from concourse import mybir
from concourse.bass_utils import run_bass_kernel_spmd

BF16 = ml_dtypes.bfloat16

# Model dims (hardcoded per spec nn_ABSA_Lstm_377957122440)
VOCAB, TVOCAB, D, H, C, B, T = 100000, 2000, 300, 300, 3, 256, 128
NCORES = 8
BL = B // NCORES          # 32 local batch
KT = 3                    # k-tiles over D/H: rows (128, 128, 44[+1 bias])
KW = (128, 128, 44)       # valid contraction rows per k-tile (H side)
KWX = (128, 128, 45)      # x side: k=2 carries the bias ones-row at row 44
NMC = 12                  # 4 gates x 3 hd-chunks of (128, 128, 44) rows
LAG = 1                   # p1-direct lookahead (steps)

_cache = {}


def _build_graph(legalize=True, debug=False, t_steps=T, reps=1,
                 trace_sim=False, variant=4):
    nc = bass.Bass()
    f32 = mybir.dt.float32
    bf16 = mybir.dt.bfloat16

    # ---- DRAM I/O (everything feature-major / transposed) ----
    xT = nc.dram_tensor("xT", [128, KT, T, BL], bf16, kind="ExternalInput")
    wihS = nc.dram_tensor("wihS", [128, KT, NMC, 128], bf16, kind="ExternalInput")
    whhS = nc.dram_tensor("whhS", [128, KT, NMC, 128], bf16, kind="ExternalInput")
    woutS = nc.dram_tensor("woutS", [128, KT, 8], bf16, kind="ExternalInput")
    boutT = nc.dram_tensor("boutT", [8, BL], f32, kind="ExternalInput")
    h0T = nc.dram_tensor("h0T", [128, KT, BL], bf16, kind="ExternalInput")
    c0T = nc.dram_tensor("c0T", [128, KT, BL], bf16, kind="ExternalInput")
    mrep = nc.dram_tensor("mrep", [128, KT, T, BL], bf16, kind="ExternalInput")
    outT = nc.dram_tensor("outT", [C, BL], f32, kind="ExternalOutput")
    dbg = None
    if debug:
        dbg = {
            "dbg_sig": nc.dram_tensor("dbg_sig", [128, NMC, BL], bf16,
                                      kind="ExternalOutput"),
            "dbg_h": nc.dram_tensor("dbg_h", [128, KT, BL], bf16,
                                    kind="ExternalOutput"),
        }

    with tile.TileContext(nc, trace_sim=trace_sim) as tc:
        for _ in range(reps):
            _body(nc, tc, xT, wihS, whhS, woutS, boutT, h0T, c0T, mrep,
                  outT, dbg, t_steps, variant)
    if legalize:
        _legalize_single_wait(nc)
    return nc


def _legalize_single_wait(nc):
    """This walrus build accepts at most ONE sync wait per instruction.
    Hoist extra waits emitted by Tile onto standalone EventSemaphore
    instructions placed immediately before the offender on the same engine."""
    for fn in nc.m.functions:
        for b in fn.blocks:
            out = []
            for inst in b.instructions:
                si = getattr(inst, "sync_info", None)
                if si is not None and si.on_wait and len(si.on_wait) > 1:
                    for w in si.on_wait[:-1]:
                        out.append(mybir.InstEventSemaphore(
                            name=nc.get_next_instruction_name(),
                            engine=inst.engine,
                            ins=[], outs=[],
                            sync_info=mybir.SyncInfo(on_wait=[w], on_update=[]),
                        ))
                    si.on_wait = [si.on_wait[-1]]
                out.append(inst)
            b.instructions[:] = out


def _body(nc, tc, xT, wihS, whhS, woutS, boutT, h0T, c0T, mrep, outT,
          dbg=None, t_steps=T, variant=4):
    f32 = mybir.dt.float32
    bf16 = mybir.dt.bfloat16
    Sig = mybir.ActivationFunctionType.Sigmoid
    Tanh = mybir.ActivationFunctionType.Tanh
    MUL = mybir.AluOpType.mult
    ADD = mybir.AluOpType.add

    from contextlib import ExitStack

    with ExitStack() as ctx:
        singles = ctx.enter_context(tc.tile_pool(name="singles", bufs=1))

        # ---- resident tiles ----
        whh_sb = singles.tile([128, KT, NMC, 128], bf16)
        nc.sync.dma_start(out=whh_sb, in_=whhS[:, :, :, :])
        wih_sb = singles.tile([128, KT, NMC, 128], bf16)
        nc.sync.dma_start(out=wih_sb, in_=wihS[:, :, :, :])
        xT_sb = singles.tile([128, KT, T, BL], bf16)
        nc.sync.dma_start(out=xT_sb, in_=xT[:, :, :, :])
        mrep_sb = singles.tile([128, KT, T, BL], bf16)
        nc.sync.dma_start(out=mrep_sb, in_=mrep[:, :, :, :])

        cT = singles.tile([128, KT, BL], bf16)
        nc.gpsimd.dma_start(out=cT, in_=c0T[:, :, :])
        HfT = singles.tile([128, KT, BL], bf16)
        nc.vector.memset(HfT, 0.0)
        hT = [singles.tile([128, KT, BL], bf16, name=f"hT{i}", tag=f"hT{i}")
              for i in range(2)]
        nc.vector.memset(hT[1], 0.0)
        nc.sync.dma_start(out=hT[0], in_=h0T[:, :, :])

        with ExitStack() as p2:
            pgpool = p2.enter_context(tc.tile_pool(name="pgp", bufs=3, space="PSUM"))
            ew = p2.enter_context(tc.tile_pool(name="ew", bufs=2))

            pg_tiles = {}

            def p1_direct(t):
                # input projection straight into step t's PSUM bank
                pg = pgpool.tile([128, NMC, BL], f32, name=f"pg{t}", tag="pg")
                pg_tiles[t] = pg
                # start=True ONLY on the first matmul touching this bank: the
                # PSUM pending-zero region is the whole 2KB bank, so each
                # byte zeroes on its first write and accumulates after.
                for mc in range(NMC):
                    for k in range(KT):
                        nc.tensor.matmul(
                            pg[:, mc, :],
                            lhsT=wih_sb[0:KWX[k], k, mc, :],
                            rhs=xT_sb[0:KWX[k], k, t, :],
                            start=(mc == 0 and k == 0), stop=False,
                            skip_group_check=True,
                        )

            for t in range(min(LAG + 1, t_steps)):
                p1_direct(t)

            for t in range(t_steps):
                cur, nxt = hT[t % 2], hT[(t + 1) % 2]
                pg = pg_tiles.pop(t)

                for mc in range(NMC):
                    for k in range(KT):
                        nc.tensor.matmul(
                            pg[:, mc, :],
                            lhsT=whh_sb[0:KW[k], k, mc, :],
                            rhs=cur[0:KW[k], k, :],
                            start=False,
                            stop=(mc == NMC - 1 and k == KT - 1),
                            skip_group_check=True,
                        )
                # prefetch next step's input projection (keeps PE warm
                # through the elementwise chain)
                if t + LAG + 1 < t_steps:
                    p1_direct(t + LAG + 1)

                if variant < 2:
                    continue

                # ONE sigmoid over all gates: sigT[:, 3g:3g+3, :]
                sigT = ew.tile([128, NMC, BL], bf16)
                nc.scalar.activation(sigT, pg[:, :, :], Sig)
                if dbg is not None and t == 0:
                    nc.sync.dma_start(out=dbg["dbg_sig"][:, :, :], in_=sigT)

                if variant < 3:
                    continue
                # tanh(g) = 2*sigma(2g) - 1 (g rows host-doubled)
                tgT = ew.tile([128, KT, BL], bf16)
                nc.vector.tensor_scalar(tgT, sigT[:, 0:3, :], 2.0, -1.0,
                                        MUL, ADD)
                fcT = ew.tile([128, KT, BL], bf16)
                nc.gpsimd.tensor_mul(fcT, sigT[:, 3:6, :], cT)
                igT = ew.tile([128, KT, BL], bf16)
                nc.vector.tensor_mul(igT, tgT, sigT[:, 6:9, :])
                nc.vector.tensor_add(cT, igT, fcT)

                # h = o * tanh(c) -> written directly as next h^T stationary
                thT = ew.tile([128, KT, BL], bf16)
                nc.scalar.activation(thT, cT, Tanh)
                nc.vector.tensor_mul(nxt, sigT[:, 9:12, :], thT)

                if dbg is not None and t == 0:
                    nc.sync.dma_start(out=dbg["dbg_h"][:, :, :], in_=nxt)

                if variant < 4:
                    continue
                # ragged capture: Hf^T += h^T * mlast-broadcast (one-hot in t)
                hmT = ew.tile([128, KT, BL], bf16)
                nc.gpsimd.tensor_mul(hmT, nxt, mrep_sb[:, :, t, :])
                nc.gpsimd.tensor_add(HfT, HfT, hmT)

        # ================= phase 3: out^T = WoutT^T-chunks @ Hf^T ==========
        with ExitStack() as p3:
            fp = p3.enter_context(tc.tile_pool(name="fp", bufs=1))
            fps = p3.enter_context(tc.tile_pool(name="fps", bufs=1, space="PSUM"))
            wout_sb = fp.tile([128, KT, 8], bf16)
            nc.sync.dma_start(out=wout_sb, in_=woutS[:, :, :])
            bout_sb = fp.tile([8, BL], f32)
            nc.sync.dma_start(out=bout_sb, in_=boutT[:, :])
            po = fps.tile([8, BL], f32)
            for k in range(KT):
                nc.tensor.matmul(po[0:C, :], lhsT=wout_sb[0:KW[k], k, 0:C],
                                 rhs=HfT[0:KW[k], k, :],
                                 start=(k == 0), stop=(k == KT - 1))
            o_sb = fp.tile([C, BL], f32)
            nc.vector.tensor_add(o_sb, po[0:C, :], bout_sb[0:C, :])
            nc.sync.dma_start(out=outT[:, :], in_=o_sb)


def _prep_inputs(sent, target, lens, emb, Wih, Whh, b_ih, b_hh, h0, c0,
                 Wout, bout):
    """Host-side shard + layout packing (data movement / tiny reindexing only)."""
    # permute gate order (i,f,g,o) -> (g,f,i,o); double g rows for the
    # tanh(g) = 2*sigmoid(2g) - 1 trick.
    perm = np.concatenate([np.arange(600, 900), np.arange(300, 600),
                           np.arange(0, 300), np.arange(900, 1200)])
    wih_r = Wih[perm].astype(np.float32)          # [1200, 300]
    whh_r = Whh[perm].astype(np.float32)
    bias_r = (b_ih + b_hh)[perm].astype(np.float32)
    wih_r[0:300] *= 2.0
    whh_r[0:300] *= 2.0
    bias_r[0:300] *= 2.0

    # stationaries: [k-rows, k, mc, m-cols], zero-padded m-cols
    wihS = np.zeros((128, KT, NMC, 128), np.float32)
    whhS = np.zeros((128, KT, NMC, 128), np.float32)
    for mc in range(NMC):
        g = mc // 3
        lo = 300 * g + 128 * (mc % 3)
        hi = min(300 * (g + 1), lo + 128)
        for k in range(KT):
            klo, khi = 128 * k, min(128 * (k + 1), D)
            wihS[0:khi - klo, k, mc, 0:hi - lo] = wih_r[lo:hi, klo:khi].T
            whhS[0:khi - klo, k, mc, 0:hi - lo] = whh_r[lo:hi, klo:khi].T
        wihS[44, 2, mc, 0:hi - lo] = bias_r[lo:hi]     # ones-row partner
    woutS = np.zeros((128, KT, 8), np.float32)
    for k in range(KT):
        klo, khi = 128 * k, min(128 * (k + 1), H)
        woutS[0:khi - klo, k, 0:C] = Wout[:, klo:khi].T

    in_maps = []
    for ci in range(NCORES):
        sl = slice(ci * BL, (ci + 1) * BL)
        x = emb[sent[sl]].astype(np.float32)       # [32, 128, 300] gather
        xT = np.zeros((128, KT, T, BL), np.float32)
        for k in range(KT):
            klo, khi = 128 * k, min(128 * (k + 1), D)
            xT[0:khi - klo, k, :, :] = x[:, :, klo:khi].transpose(2, 1, 0)
        xT[44, 2, :, :] = 1.0                      # bias ones-row

        h0T = np.zeros((128, KT, BL), np.float32)
        c0T = np.zeros((128, KT, BL), np.float32)
        for k in range(KT):
            klo, khi = 128 * k, min(128 * (k + 1), H)
            h0T[0:khi - klo, k, :] = h0[sl, klo:khi].T
            c0T[0:khi - klo, k, :] = c0[sl, klo:khi].T

        lloc = lens[sl].astype(np.int64)
        mlast = np.zeros((BL, T), np.float32)
        mlast[np.arange(BL), np.clip(lloc - 1, 0, T - 1)] = 1.0
        mrep = np.broadcast_to(mlast.T[None, None, :, :],
                               (128, KT, T, BL)).copy()

        in_maps.append({
            "xT": xT.astype(BF16),
            "wihS": wihS.astype(BF16),
            "whhS": whhS.astype(BF16),
            "woutS": woutS.astype(BF16),
            "boutT": np.pad(np.tile(bout.astype(np.float32)[:, None],
                                    (1, BL)), ((0, 8 - C), (0, 0))),
            "h0T": h0T.astype(BF16),
            "c0T": c0T.astype(BF16),
            "mrep": mrep.astype(BF16),
        })
    return in_maps


def kernel(**inputs):
    if "nc" not in _cache:
        _cache["nc"] = _build_graph()
    nc = _cache["nc"]
    in_maps = _prep_inputs(**inputs)
    res = run_bass_kernel_spmd(nc, in_maps, core_ids=list(range(NCORES)))
    outs = [res.results[i]["outT"].T for i in range(NCORES)]
    return np.concatenate(outs, axis=0).astype(np.float32)


# revision 16
# speedup vs baseline: 2.3080x; 1.0703x over previous
"""
Trainium2 Bass kernel for nn_ABSA_Lstm: masked LSTM over ragged sequences.

  reference:  x = emb[sent]; LSTM over T=128 steps with per-sequence length
              masking; out = h_final @ Wout.T + bout   -> [256, 3]

Strategy (8 NeuronCores, data parallel):
  - Shard batch B=256 -> 32 sequences per core. LSTM weights replicated.
  - Host does the embedding-table gather (pure data movement) and packs
    transposed/padded tile layouts; all model FLOPs run on device.

FULLY TRANSPOSED formulation: every on-chip tensor carries feature dims on
PARTITIONS and batch on the FREE dim.  Engine cost on TRN2 is free-size
driven, so with batch (32) as the free dim every instruction is near its
fixed-cost floor, and:
  - gates^T [1200, 32] accumulate in ONE PSUM bank as [128p, 12 chunks, 32]
    (per-gate hd-chunks of 128/128/44 rows; gate order g,f,i,o; stationary
    weight columns are zero-padded to 128 so no partition garbage exists);
  - the input projection runs DIRECTLY into each step's PSUM bank
    (start=True) from resident xT tiles -- no gx buffer, no inject matmuls,
    no PSUM->SBUF drain;
  - h^T is produced by the elementwise chain already transposed -- the
    per-step PE transpose + copy of the h-major formulation disappears;
  - ONE sigmoid covers all 4 gates (host pre-doubles the g-gate rows;
    tanh(g) = 2*sigmoid(2g)-1 via a single 2x-mode tensor_scalar);
  - all elementwise ops are 2x-capable TensorTensor/TensorScalar at
    partition base 0 (no start-partition pairing issues);
  - ragged lengths: recurrence runs unmasked, Hf^T += h^T * mrep[:, :, t, :]
    with a one-hot-in-t replicated mask (two Pool ops, off critical path).
  - bias rides a ones-row in the xT k=2 tile (row 44) against a bias row in
    the Wih stationary; biases cost zero instructions.

This walrus build accepts only ONE sync wait per instruction, so after Tile
scheduling, extra waits are hoisted onto standalone EventSemaphore
instructions (_legalize_single_wait).
"""

import numpy as np
import ml_dtypes

import concourse.bass as bass
import concourse.tile as tile
from concourse import mybir
from concourse.bass_utils import run_bass_kernel_spmd

BF16 = ml_dtypes.bfloat16

# Model dims (hardcoded per spec nn_ABSA_Lstm_377957122440)
VOCAB, TVOCAB, D, H, C, B, T = 100000, 2000, 300, 300, 3, 256, 128
NCORES = 8
BL = B // NCORES          # 32 local batch
KT = 3                    # k-tiles over D/H: rows (128, 128, 44[+1 bias])
KW = (128, 128, 44)       # valid contraction rows per k-tile (H side)
KWX = (128, 128, 45)      # x side: k=2 carries the bias ones-row at row 44
NMC = 12                  # 4 gates x 3 hd-chunks of (128, 128, 44) rows
LAG = 1                   # p1-direct lookahead (steps)

_cache = {}


def _build_graph(legalize=True, debug=False, t_steps=T, reps=1,
                 trace_sim=False, variant=4):
    nc = bass.Bass()
    f32 = mybir.dt.float32
    bf16 = mybir.dt.bfloat16

    # ---- DRAM I/O (everything feature-major / transposed) ----
    xT = nc.dram_tensor("xT", [128, KT, T, BL], bf16, kind="ExternalInput")
    wihS = nc.dram_tensor("wihS", [128, KT, NMC, 128], bf16, kind="ExternalInput")
    whhS = nc.dram_tensor("whhS", [128, KT, NMC, 128], bf16, kind="ExternalInput")
    woutS = nc.dram_tensor("woutS", [128, KT, 8], bf16, kind="ExternalInput")
    boutT = nc.dram_tensor("boutT", [8, BL], f32, kind="ExternalInput")
    h0T = nc.dram_tensor("h0T", [128, KT, BL], bf16, kind="ExternalInput")
    c0T = nc.dram_tensor("c0T", [128, KT, BL], bf16, kind="ExternalInput")
    mrep = nc.dram_tensor("mrep", [128, KT, T, BL], bf16, kind="ExternalInput")
    outT = nc.dram_tensor("outT", [C, BL], f32, kind="ExternalOutput")
    dbg = None
    if debug:
        dbg = {
            "dbg_sig": nc.dram_tensor("dbg_sig", [128, NMC, BL], bf16,
                                      kind="ExternalOutput"),
            "dbg_h": nc.dram_tensor("dbg_h", [128, KT, BL], bf16,
                                    kind="ExternalOutput"),
        }

    with tile.TileContext(nc, trace_sim=trace_sim) as tc:
        for _ in range(reps):
            _body(nc, tc, xT, wihS, whhS, woutS, boutT, h0T, c0T, mrep,
                  outT, dbg, t_steps, variant)
    if legalize:
        _legalize_single_wait(nc)
    return nc


def _legalize_single_wait(nc):
    """This walrus build accepts at most ONE sync wait per instruction.
    Hoist extra waits emitted by Tile onto standalone EventSemaphore
    instructions placed immediately before the offender on the same engine."""
    for fn in nc.m.functions:
        for b in fn.blocks:
            out = []
            for inst in b.instructions:
                si = getattr(inst, "sync_info", None)
                if si is not None and si.on_wait and len(si.on_wait) > 1:
                    for w in si.on_wait[:-1]:
                        out.append(mybir.InstEventSemaphore(
                            name=nc.get_next_instruction_name(),
                            engine=inst.engine,
                            ins=[], outs=[],
                            sync_info=mybir.SyncInfo(on_wait=[w], on_update=[]),
                        ))
                    si.on_wait = [si.on_wait[-1]]
                out.append(inst)
            b.instructions[:] = out


def _body(nc, tc, xT, wihS, whhS, woutS, boutT, h0T, c0T, mrep, outT,
          dbg=None, t_steps=T, variant=4):
    f32 = mybir.dt.float32
    bf16 = mybir.dt.bfloat16
    Sig = mybir.ActivationFunctionType.Sigmoid
    Tanh = mybir.ActivationFunctionType.Tanh
    MUL = mybir.AluOpType.mult
    ADD = mybir.AluOpType.add

    from contextlib import ExitStack

    with ExitStack() as ctx:
        singles = ctx.enter_context(tc.tile_pool(name="singles", bufs=1))

        # ---- resident tiles ----
        whh_sb = singles.tile([128, KT, NMC, 128], bf16)
        nc.sync.dma_start(out=whh_sb, in_=whhS[:, :, :, :])
        wih_sb = singles.tile([128, KT, NMC, 128], bf16)
        nc.sync.dma_start(out=wih_sb, in_=wihS[:, :, :, :])
        xT_sb = singles.tile([128, KT, T, BL], bf16)
        nc.sync.dma_start(out=xT_sb, in_=xT[:, :, :, :])
        mrep_sb = singles.tile([128, KT, T, BL], bf16)
        nc.sync.dma_start(out=mrep_sb, in_=mrep[:, :, :, :])

        cT = singles.tile([128, KT, BL], bf16)
        nc.gpsimd.dma_start(out=cT, in_=c0T[:, :, :])
        HfT = singles.tile([128, KT, BL], bf16)
        nc.vector.memset(HfT, 0.0)
        hT = [singles.tile([128, KT, BL], bf16, name=f"hT{i}", tag=f"hT{i}")
              for i in range(2)]
        nc.vector.memset(hT[1], 0.0)
        nc.sync.dma_start(out=hT[0], in_=h0T[:, :, :])

        with ExitStack() as p2:
            pgpool = p2.enter_context(tc.tile_pool(name="pgp", bufs=2, space="PSUM"))
            popool = p2.enter_context(tc.tile_pool(name="pop", bufs=2, space="PSUM"))
            ew = p2.enter_context(tc.tile_pool(name="ew", bufs=2))

            pg_tiles = {}

            def p1_direct(t):
                # input projection straight into step t's PSUM banks.
                # Gate chunks: f 0-2, g 3-5, i 6-8 in pg; o 0-2 in po (its
                # own bank so sigma(f,g,i) doesn't wait on o's matmuls:
                # dependencies are tile-granular).
                # start=True ONLY on the first matmul touching each bank:
                # the PSUM pending-zero region is the whole 2KB bank, so
                # each byte zeroes on its first write and accumulates after.
                pg = pgpool.tile([128, 9, BL], f32, name=f"pg{t}", tag="pg")
                po = popool.tile([128, KT, BL], f32, name=f"po{t}", tag="po")
                pg_tiles[t] = (pg, po)
                for mc in range(NMC):
                    dst = pg[:, mc, :] if mc < 9 else po[:, mc - 9, :]
                    for k in range(KT):
                        nc.tensor.matmul(
                            dst,
                            lhsT=wih_sb[0:KWX[k], k, mc, :],
                            rhs=xT_sb[0:KWX[k], k, t, :],
                            start=(k == 0 and mc % 9 == 0), stop=False,
                            skip_group_check=True,
                        )

            for t in range(min(LAG + 1, t_steps)):
                p1_direct(t)

            for t in range(t_steps):
                cur, nxt = hT[t % 2], hT[(t + 1) % 2]
                pg, po = pg_tiles.pop(t)

                for mc in range(NMC):
                    dst = pg[:, mc, :] if mc < 9 else po[:, mc - 9, :]
                    for k in range(KT):
                        nc.tensor.matmul(
                            dst,
                            lhsT=whh_sb[0:KW[k], k, mc, :],
                            rhs=cur[0:KW[k], k, :],
                            start=False,
                            stop=(k == KT - 1 and mc in (8, NMC - 1)),
                            skip_group_check=True,
                        )
                # prefetch next step's input projection (keeps PE warm
                # through the elementwise chain)
                if t + LAG + 1 < t_steps:
                    p1_direct(t + LAG + 1)

                if variant < 2:
                    continue

                # sigma over (f,g,i); o's sigma runs off the critical path
                sigT = ew.tile([128, 9, BL], bf16)
                nc.scalar.activation(sigT, pg[:, :, :], Sig)
                sigO = ew.tile([128, KT, BL], bf16)
                nc.scalar.activation(sigO, po[:, :, :], Sig)
                if dbg is not None and t == 0:
                    nc.sync.dma_start(out=dbg["dbg_sig"][:, 0:9, :], in_=sigT)
                    nc.sync.dma_start(out=dbg["dbg_sig"][:, 9:12, :],
                                      in_=sigO)

                if variant < 3:
                    continue
                # whole c-chain on Pool (program order, no cross-engine
                # hops):  fc = f*c;  ig2 = (sig(2g)-0.5)*i = i*tanh(g)/2;
                # c = 2*ig2 + fc
                fcT = ew.tile([128, KT, BL], bf16)
                nc.vector.tensor_mul(fcT, sigT[:, 0:3, :], cT)
                igT = ew.tile([128, KT, BL], bf16)
                nc.vector.scalar_tensor_tensor(
                    igT, sigT[:, 3:6, :], 0.5, sigT[:, 6:9, :],
                    mybir.AluOpType.subtract, MUL)
                nc.vector.scalar_tensor_tensor(cT, igT, 2.0, fcT, MUL, ADD)

                # h = o * tanh(c) -> written directly as next h^T stationary
                thT = ew.tile([128, KT, BL], bf16)
                nc.scalar.activation(thT, cT, Tanh)
                nc.vector.tensor_mul(nxt, sigO, thT)

                if dbg is not None and t == 0:
                    nc.sync.dma_start(out=dbg["dbg_h"][:, :, :], in_=nxt)

                if variant < 4:
                    continue
                # ragged capture: Hf^T += h^T * mlast-broadcast (one-hot in t)
                hmT = ew.tile([128, KT, BL], bf16)
                nc.gpsimd.tensor_mul(hmT, nxt, mrep_sb[:, :, t, :])
                nc.gpsimd.tensor_add(HfT, HfT, hmT)

        # ================= phase 3: out^T = WoutT^T-chunks @ Hf^T ==========
        with ExitStack() as p3:
            fp = p3.enter_context(tc.tile_pool(name="fp", bufs=1))
            fps = p3.enter_context(tc.tile_pool(name="fps", bufs=1, space="PSUM"))
            wout_sb = fp.tile([128, KT, 8], bf16)
            nc.sync.dma_start(out=wout_sb, in_=woutS[:, :, :])
            bout_sb = fp.tile([8, BL], f32)
            nc.sync.dma_start(out=bout_sb, in_=boutT[:, :])
            po = fps.tile([8, BL], f32)
            for k in range(KT):
                nc.tensor.matmul(po[0:C, :], lhsT=wout_sb[0:KW[k], k, 0:C],
                                 rhs=HfT[0:KW[k], k, :],
                                 start=(k == 0), stop=(k == KT - 1))
            o_sb = fp.tile([C, BL], f32)
            nc.vector.tensor_add(o_sb, po[0:C, :], bout_sb[0:C, :])
            nc.sync.dma_start(out=outT[:, :], in_=o_sb)


def _prep_inputs(sent, target, lens, emb, Wih, Whh, b_ih, b_hh, h0, c0,
                 Wout, bout):
    """Host-side shard + layout packing (data movement / tiny reindexing only)."""
    # permute gate order (i,f,g,o) -> (f,g,i,o); double g rows for the
    # tanh(g) = 2*sigmoid(2g) - 1 trick.
    perm = np.concatenate([np.arange(300, 600), np.arange(600, 900),
                           np.arange(0, 300), np.arange(900, 1200)])
    wih_r = Wih[perm].astype(np.float32)          # [1200, 300]
    whh_r = Whh[perm].astype(np.float32)
    bias_r = (b_ih + b_hh)[perm].astype(np.float32)
    wih_r[300:600] *= 2.0
    whh_r[300:600] *= 2.0
    bias_r[300:600] *= 2.0

    # stationaries: [k-rows, k, mc, m-cols], zero-padded m-cols
    wihS = np.zeros((128, KT, NMC, 128), np.float32)
    whhS = np.zeros((128, KT, NMC, 128), np.float32)
    for mc in range(NMC):
        g = mc // 3
        lo = 300 * g + 128 * (mc % 3)
        hi = min(300 * (g + 1), lo + 128)
        for k in range(KT):
            klo, khi = 128 * k, min(128 * (k + 1), D)
            wihS[0:khi - klo, k, mc, 0:hi - lo] = wih_r[lo:hi, klo:khi].T
            whhS[0:khi - klo, k, mc, 0:hi - lo] = whh_r[lo:hi, klo:khi].T
        wihS[44, 2, mc, 0:hi - lo] = bias_r[lo:hi]     # ones-row partner
    woutS = np.zeros((128, KT, 8), np.float32)
    for k in range(KT):
        klo, khi = 128 * k, min(128 * (k + 1), H)
        woutS[0:khi - klo, k, 0:C] = Wout[:, klo:khi].T

    in_maps = []
    for ci in range(NCORES):
        sl = slice(ci * BL, (ci + 1) * BL)
        x = emb[sent[sl]].astype(np.float32)       # [32, 128, 300] gather
        xT = np.zeros((128, KT, T, BL), np.float32)
        for k in range(KT):
            klo, khi = 128 * k, min(128 * (k + 1), D)
            xT[0:khi - klo, k, :, :] = x[:, :, klo:khi].transpose(2, 1, 0)
        xT[44, 2, :, :] = 1.0                      # bias ones-row

        h0T = np.zeros((128, KT, BL), np.float32)
        c0T = np.zeros((128, KT, BL), np.float32)
        for k in range(KT):
            klo, khi = 128 * k, min(128 * (k + 1), H)
            h0T[0:khi - klo, k, :] = h0[sl, klo:khi].T
            c0T[0:khi - klo, k, :] = c0[sl, klo:khi].T

        lloc = lens[sl].astype(np.int64)
        mlast = np.zeros((BL, T), np.float32)
        mlast[np.arange(BL), np.clip(lloc - 1, 0, T - 1)] = 1.0
        mrep = np.broadcast_to(mlast.T[None, None, :, :],
                               (128, KT, T, BL)).copy()

        in_maps.append({
            "xT": xT.astype(BF16),
            "wihS": wihS.astype(BF16),
            "whhS": whhS.astype(BF16),
            "woutS": woutS.astype(BF16),
            "boutT": np.pad(np.tile(bout.astype(np.float32)[:, None],
                                    (1, BL)), ((0, 8 - C), (0, 0))),
            "h0T": h0T.astype(BF16),
            "c0T": c0T.astype(BF16),
            "mrep": mrep.astype(BF16),
        })
    return in_maps


def kernel(**inputs):
    if "nc" not in _cache:
        _cache["nc"] = _build_graph()
    nc = _cache["nc"]
    in_maps = _prep_inputs(**inputs)
    res = run_bass_kernel_spmd(nc, in_maps, core_ids=list(range(NCORES)))
    outs = [res.results[i]["outT"].T for i in range(NCORES)]
    return np.concatenate(outs, axis=0).astype(np.float32)


# revision 19
# speedup vs baseline: 2.7553x; 1.1938x over previous
"""
Trainium2 Bass kernel for nn_ABSA_Lstm: masked LSTM over ragged sequences.

  reference:  x = emb[sent]; LSTM over T=128 steps with per-sequence length
              masking; out = h_final @ Wout.T + bout   -> [256, 3]

Strategy (8 NeuronCores, data parallel):
  - Shard batch B=256 -> 32 sequences per core. LSTM weights replicated.
  - Host does the embedding-table gather (pure data movement) and packs
    transposed/padded tile layouts; all model FLOPs run on device.

FULLY TRANSPOSED formulation: every on-chip tensor carries feature dims on
PARTITIONS and batch on the FREE dim.  Engine cost on TRN2 is free-size
driven, so with batch (32) as the free dim every instruction is near its
fixed-cost floor, and:
  - gates^T [1200, 32] accumulate in ONE PSUM bank as [128p, 12 chunks, 32]
    (per-gate hd-chunks of 128/128/44 rows; gate order g,f,i,o; stationary
    weight columns are zero-padded to 128 so no partition garbage exists);
  - the input projection runs DIRECTLY into each step's PSUM bank
    (start=True) from resident xT tiles -- no gx buffer, no inject matmuls,
    no PSUM->SBUF drain;
  - h^T is produced by the elementwise chain already transposed -- the
    per-step PE transpose + copy of the h-major formulation disappears;
  - ONE sigmoid covers all 4 gates (host pre-doubles the g-gate rows;
    tanh(g) = 2*sigmoid(2g)-1 via a single 2x-mode tensor_scalar);
  - all elementwise ops are 2x-capable TensorTensor/TensorScalar at
    partition base 0 (no start-partition pairing issues);
  - ragged lengths: recurrence runs unmasked, Hf^T += h^T * mrep[:, :, t, :]
    with a one-hot-in-t replicated mask (two Pool ops, off critical path).
  - bias rides a ones-row in the xT k=2 tile (row 44) against a bias row in
    the Wih stationary; biases cost zero instructions.

This walrus build accepts only ONE sync wait per instruction, so after Tile
scheduling, extra waits are hoisted onto standalone EventSemaphore
instructions (_legalize_single_wait).
"""

import numpy as np
import ml_dtypes

import concourse.bass as bass
import concourse.tile as tile
from concourse import mybir
from concourse.bass_utils import run_bass_kernel_spmd

BF16 = ml_dtypes.bfloat16

# Model dims (hardcoded per spec nn_ABSA_Lstm_377957122440)
VOCAB, TVOCAB, D, H, C, B, T = 100000, 2000, 300, 300, 3, 256, 128
NCORES = 8
BL = B // NCORES          # 32 local batch
KT = 3                    # k-tiles over D/H: rows (128, 128, 44[+1 bias])
KW = (128, 128, 44)       # valid contraction rows per k-tile (H side)
KWX = (128, 128, 45)      # x side: k=2 carries the bias ones-row at row 44
NMC = 12                  # 4 gates x 3 hd-chunks of (128, 128, 44) rows
LAG = 1                   # p1-direct lookahead (steps)

_cache = {}


def _build_graph(legalize=True, debug=False, t_steps=T, reps=1,
                 trace_sim=False, variant=4):
    nc = bass.Bass()
    f32 = mybir.dt.float32
    bf16 = mybir.dt.bfloat16

    # ---- DRAM I/O (everything feature-major / transposed) ----
    xT = nc.dram_tensor("xT", [128, KT, T, BL], bf16, kind="ExternalInput")
    wihS = nc.dram_tensor("wihS", [128, KT, NMC, 128], bf16, kind="ExternalInput")
    whhS = nc.dram_tensor("whhS", [128, KT, NMC, 128], bf16, kind="ExternalInput")
    woutS = nc.dram_tensor("woutS", [128, KT, 8], bf16, kind="ExternalInput")
    boutT = nc.dram_tensor("boutT", [8, BL], f32, kind="ExternalInput")
    h0T = nc.dram_tensor("h0T", [128, KT, BL], bf16, kind="ExternalInput")
    c0T = nc.dram_tensor("c0T", [128, KT, BL], bf16, kind="ExternalInput")
    mrep = nc.dram_tensor("mrep", [128, KT, T, BL], bf16, kind="ExternalInput")
    outT = nc.dram_tensor("outT", [C, BL], f32, kind="ExternalOutput")
    dbg = None
    if debug:
        dbg = {
            "dbg_sig": nc.dram_tensor("dbg_sig", [128, NMC, BL], bf16,
                                      kind="ExternalOutput"),
            "dbg_h": nc.dram_tensor("dbg_h", [128, KT, BL], bf16,
                                    kind="ExternalOutput"),
        }

    with tile.TileContext(nc, trace_sim=trace_sim) as tc:
        for _ in range(reps):
            _body(nc, tc, xT, wihS, whhS, woutS, boutT, h0T, c0T, mrep,
                  outT, dbg, t_steps, variant)
    if legalize:
        _legalize_single_wait(nc)
    return nc


def _legalize_single_wait(nc):
    """This walrus build accepts at most ONE sync wait per instruction.
    Hoist extra waits emitted by Tile onto standalone EventSemaphore
    instructions placed immediately before the offender on the same engine."""
    for fn in nc.m.functions:
        for b in fn.blocks:
            out = []
            for inst in b.instructions:
                si = getattr(inst, "sync_info", None)
                if si is not None and si.on_wait and len(si.on_wait) > 1:
                    for w in si.on_wait[:-1]:
                        out.append(mybir.InstEventSemaphore(
                            name=nc.get_next_instruction_name(),
                            engine=inst.engine,
                            ins=[], outs=[],
                            sync_info=mybir.SyncInfo(on_wait=[w], on_update=[]),
                        ))
                    si.on_wait = [si.on_wait[-1]]
                out.append(inst)
            b.instructions[:] = out


def _body(nc, tc, xT, wihS, whhS, woutS, boutT, h0T, c0T, mrep, outT,
          dbg=None, t_steps=T, variant=4):
    f32 = mybir.dt.float32
    bf16 = mybir.dt.bfloat16
    Sig = mybir.ActivationFunctionType.Sigmoid
    Tanh = mybir.ActivationFunctionType.Tanh
    MUL = mybir.AluOpType.mult
    ADD = mybir.AluOpType.add

    from contextlib import ExitStack

    with ExitStack() as ctx:
        singles = ctx.enter_context(tc.tile_pool(name="singles", bufs=1))

        # ---- resident tiles (wih first: p1d(0) gates the loop start; xT
        # quartered so step 0 only waits 1/4 of its DMA; mrep on another
        # queue, only the off-path capture reads it) ----
        wih_sb = singles.tile([128, KT, NMC, 128], bf16)
        nc.sync.dma_start(out=wih_sb, in_=wihS[:, :, :, :])
        NQ, TQ = 4, T // 4
        xT_q = [singles.tile([128, KT, TQ, BL], bf16, name=f"xq{q}",
                             tag=f"xq{q}") for q in range(NQ)]
        for q in range(NQ):
            nc.sync.dma_start(out=xT_q[q],
                              in_=xT[:, :, q * TQ:(q + 1) * TQ, :])
        whh_sb = singles.tile([128, KT, NMC, 128], bf16)
        nc.sync.dma_start(out=whh_sb, in_=whhS[:, :, :, :])
        mrep_sb = singles.tile([128, KT, T, BL], bf16)
        nc.scalar.dma_start(out=mrep_sb, in_=mrep[:, :, :, :])

        HB = BL // 2          # batch halves: two independent pipelines
        cT = [[singles.tile([128, KT, HB], bf16, name=f"cT{h}_{i}",
                            tag=f"cT{h}_{i}") for i in range(2)]
              for h in range(2)]
        HfT = singles.tile([128, KT, BL], bf16)
        nc.vector.memset(HfT, 0.0)
        hT = [[singles.tile([128, KT, HB], bf16, name=f"hT{h}_{i}",
                            tag=f"hT{h}_{i}") for i in range(2)]
              for h in range(2)]
        for h in range(2):
            bs = slice(h * HB, (h + 1) * HB)
            nc.gpsimd.dma_start(out=cT[h][0], in_=c0T[:, :, bs])
            nc.gpsimd.dma_start(out=hT[h][0], in_=h0T[:, :, bs])
            nc.vector.memset(hT[h][1], 0.0)

        with ExitStack() as p2:
            pgpool = [p2.enter_context(
                tc.tile_pool(name=f"pgp{h}", bufs=2, space="PSUM"))
                for h in range(2)]
            popool = [p2.enter_context(
                tc.tile_pool(name=f"pop{h}", bufs=2, space="PSUM"))
                for h in range(2)]
            ew = p2.enter_context(tc.tile_pool(name="ew", bufs=2))

            pg_tiles = {}

            def p1_direct(t):
                # input projection straight into step t's PSUM banks.
                # Gate chunks: f 0-2, g 3-5, i 6-8 in pg; o 0-2 in po (its
                # own bank so sigma(f,g,i) doesn't wait on o's matmuls:
                # dependencies are tile-granular).
                # start=True ONLY on the first matmul touching each bank:
                # the PSUM pending-zero region is the whole 2KB bank, so
                # each byte zeroes on its first write and accumulates after.
                pgs = [pgpool[h].tile([128, 9, HB], f32, name=f"pg{t}_{h}",
                                      tag=f"pg{h}") for h in range(2)]
                pos = [popool[h].tile([128, KT, HB], f32, name=f"po{t}_{h}",
                                      tag=f"po{h}") for h in range(2)]
                pg_tiles[t] = (pgs, pos)
                xq = xT_q[t // TQ]
                for h in range(2):
                    bs = slice(h * HB, (h + 1) * HB)
                    for mc in range(NMC):
                        dst = (pgs[h][:, mc, :] if mc < 9
                               else pos[h][:, mc - 9, :])
                        for k in range(KT):
                            nc.tensor.matmul(
                                dst,
                                lhsT=wih_sb[0:KWX[k], k, mc, :],
                                rhs=xq[0:KWX[k], k, t % TQ, bs],
                                start=(k == 0 and mc % 9 == 0), stop=False,
                                skip_group_check=True,
                            )

            for t in range(min(LAG + 1, t_steps)):
                p1_direct(t)

            for t in range(t_steps):
                pgs, pos = pg_tiles.pop(t)

                for h in range(2):
                    cur = hT[h][t % 2]
                    for mc in range(NMC):
                        dst = (pgs[h][:, mc, :] if mc < 9
                               else pos[h][:, mc - 9, :])
                        for k in range(KT):
                            nc.tensor.matmul(
                                dst,
                                lhsT=whh_sb[0:KW[k], k, mc, :],
                                rhs=cur[0:KW[k], k, :],
                                start=False,
                                stop=(k == KT - 1 and mc in (8, NMC - 1)),
                                skip_group_check=True,
                            )
                # prefetch next step's input projection (keeps PE warm
                # through the elementwise chain)
                if t + LAG + 1 < t_steps:
                    p1_direct(t + LAG + 1)

                if variant < 2:
                    continue

                # per batch-half pipelines; sigma over (f,g,i) on the
                # critical path, o's sigma off it
                sigTs, sigOs = [], []
                for h in range(2):
                    sigT = ew.tile([128, 9, HB], bf16)
                    nc.scalar.activation(sigT, pgs[h][:, :, :], Sig)
                    sigO = ew.tile([128, KT, HB], bf16)
                    nc.scalar.activation(sigO, pos[h][:, :, :], Sig)
                    sigTs.append(sigT)
                    sigOs.append(sigO)
                if dbg is not None and t == 0:
                    for h in range(2):
                        bs = slice(h * HB, (h + 1) * HB)
                        nc.sync.dma_start(out=dbg["dbg_sig"][:, 0:9, bs],
                                          in_=sigTs[h])
                        nc.sync.dma_start(out=dbg["dbg_sig"][:, 9:12, bs],
                                          in_=sigOs[h])

                if variant < 3:
                    continue
                # c-chain per half on DVE (program order, no internal hops):
                # fc = f*c; ig2 = (sig(2g)-0.5)*i = i*tanh(g)/2; c = 2*ig2+fc
                for h in range(2):
                    sigT = sigTs[h]
                    cold, cnew = cT[h][t % 2], cT[h][(t + 1) % 2]
                    fcT = ew.tile([128, KT, HB], bf16)
                    nc.vector.tensor_mul(fcT, sigT[:, 0:3, :], cold)
                    igT = ew.tile([128, KT, HB], bf16)
                    nc.vector.scalar_tensor_tensor(
                        igT, sigT[:, 3:6, :], 0.5, sigT[:, 6:9, :],
                        mybir.AluOpType.subtract, MUL)
                    nc.vector.scalar_tensor_tensor(
                        cnew, igT, 2.0, fcT, MUL, ADD)

                    # h = o * tanh(c) -> next h^T stationary directly
                    thT = ew.tile([128, KT, HB], bf16)
                    nc.scalar.activation(thT, cnew, Tanh)
                    nc.vector.tensor_mul(hT[h][(t + 1) % 2], sigOs[h], thT)

                if dbg is not None and t == 0:
                    for h in range(2):
                        bs = slice(h * HB, (h + 1) * HB)
                        nc.sync.dma_start(out=dbg["dbg_h"][:, :, bs],
                                          in_=hT[h][(t + 1) % 2])

                if variant < 4:
                    continue
                # ragged capture: Hf^T += h^T * mlast-broadcast (one-hot in t)
                for h in range(2):
                    bs = slice(h * HB, (h + 1) * HB)
                    hmT = ew.tile([128, KT, HB], bf16)
                    nc.gpsimd.tensor_mul(hmT, hT[h][(t + 1) % 2],
                                         mrep_sb[:, :, t, bs])
                    nc.gpsimd.tensor_add(HfT[:, :, bs], HfT[:, :, bs], hmT)

        # ================= phase 3: out^T = WoutT^T-chunks @ Hf^T ==========
        with ExitStack() as p3:
            fp = p3.enter_context(tc.tile_pool(name="fp", bufs=1))
            fps = p3.enter_context(tc.tile_pool(name="fps", bufs=1, space="PSUM"))
            wout_sb = fp.tile([128, KT, 8], bf16)
            nc.sync.dma_start(out=wout_sb, in_=woutS[:, :, :])
            bout_sb = fp.tile([8, BL], f32)
            nc.sync.dma_start(out=bout_sb, in_=boutT[:, :])
            po = fps.tile([8, BL], f32)
            for k in range(KT):
                nc.tensor.matmul(po[0:C, :], lhsT=wout_sb[0:KW[k], k, 0:C],
                                 rhs=HfT[0:KW[k], k, :],
                                 start=(k == 0), stop=(k == KT - 1))
            o_sb = fp.tile([C, BL], f32)
            nc.vector.tensor_add(o_sb, po[0:C, :], bout_sb[0:C, :])
            nc.sync.dma_start(out=outT[:, :], in_=o_sb)


def _prep_inputs(sent, target, lens, emb, Wih, Whh, b_ih, b_hh, h0, c0,
                 Wout, bout):
    """Host-side shard + layout packing (data movement / tiny reindexing only)."""
    # permute gate order (i,f,g,o) -> (f,g,i,o); double g rows for the
    # tanh(g) = 2*sigmoid(2g) - 1 trick.
    perm = np.concatenate([np.arange(300, 600), np.arange(600, 900),
                           np.arange(0, 300), np.arange(900, 1200)])
    wih_r = Wih[perm].astype(np.float32)          # [1200, 300]
    whh_r = Whh[perm].astype(np.float32)
    bias_r = (b_ih + b_hh)[perm].astype(np.float32)
    wih_r[300:600] *= 2.0
    whh_r[300:600] *= 2.0
    bias_r[300:600] *= 2.0

    # stationaries: [k-rows, k, mc, m-cols], zero-padded m-cols
    wihS = np.zeros((128, KT, NMC, 128), np.float32)
    whhS = np.zeros((128, KT, NMC, 128), np.float32)
    for mc in range(NMC):
        g = mc // 3
        lo = 300 * g + 128 * (mc % 3)
        hi = min(300 * (g + 1), lo + 128)
        for k in range(KT):
            klo, khi = 128 * k, min(128 * (k + 1), D)
            wihS[0:khi - klo, k, mc, 0:hi - lo] = wih_r[lo:hi, klo:khi].T
            whhS[0:khi - klo, k, mc, 0:hi - lo] = whh_r[lo:hi, klo:khi].T
        wihS[44, 2, mc, 0:hi - lo] = bias_r[lo:hi]     # ones-row partner
    woutS = np.zeros((128, KT, 8), np.float32)
    for k in range(KT):
        klo, khi = 128 * k, min(128 * (k + 1), H)
        woutS[0:khi - klo, k, 0:C] = Wout[:, klo:khi].T

    in_maps = []
    for ci in range(NCORES):
        sl = slice(ci * BL, (ci + 1) * BL)
        x = emb[sent[sl]].astype(np.float32)       # [32, 128, 300] gather
        xT = np.zeros((128, KT, T, BL), np.float32)
        for k in range(KT):
            klo, khi = 128 * k, min(128 * (k + 1), D)
            xT[0:khi - klo, k, :, :] = x[:, :, klo:khi].transpose(2, 1, 0)
        xT[44, 2, :, :] = 1.0                      # bias ones-row

        h0T = np.zeros((128, KT, BL), np.float32)
        c0T = np.zeros((128, KT, BL), np.float32)
        for k in range(KT):
            klo, khi = 128 * k, min(128 * (k + 1), H)
            h0T[0:khi - klo, k, :] = h0[sl, klo:khi].T
            c0T[0:khi - klo, k, :] = c0[sl, klo:khi].T

        lloc = lens[sl].astype(np.int64)
        mlast = np.zeros((BL, T), np.float32)
        mlast[np.arange(BL), np.clip(lloc - 1, 0, T - 1)] = 1.0
        mrep = np.broadcast_to(mlast.T[None, None, :, :],
                               (128, KT, T, BL)).copy()

        in_maps.append({
            "xT": xT.astype(BF16),
            "wihS": wihS.astype(BF16),
            "whhS": whhS.astype(BF16),
            "woutS": woutS.astype(BF16),
            "boutT": np.pad(np.tile(bout.astype(np.float32)[:, None],
                                    (1, BL)), ((0, 8 - C), (0, 0))),
            "h0T": h0T.astype(BF16),
            "c0T": c0T.astype(BF16),
            "mrep": mrep.astype(BF16),
        })
    return in_maps


def kernel(**inputs):
    if "nc" not in _cache:
        _cache["nc"] = _build_graph()
    nc = _cache["nc"]
    in_maps = _prep_inputs(**inputs)
    res = run_bass_kernel_spmd(nc, in_maps, core_ids=list(range(NCORES)))
    outs = [res.results[i]["outT"].T for i in range(NCORES)]
    return np.concatenate(outs, axis=0).astype(np.float32)


# revision 20
# speedup vs baseline: 2.8187x; 1.0230x over previous
"""
Trainium2 Bass kernel for nn_ABSA_Lstm: masked LSTM over ragged sequences.

  reference:  x = emb[sent]; LSTM over T=128 steps with per-sequence length
              masking; out = h_final @ Wout.T + bout   -> [256, 3]

Strategy (8 NeuronCores, data parallel):
  - Shard batch B=256 -> 32 sequences per core. LSTM weights replicated.
  - Host does the embedding-table gather (pure data movement) and packs
    transposed/padded tile layouts; all model FLOPs run on device.

FULLY TRANSPOSED formulation: every on-chip tensor carries feature dims on
PARTITIONS and batch on the FREE dim.  Engine cost on TRN2 is free-size
driven, so with batch (32) as the free dim every instruction is near its
fixed-cost floor, and:
  - gates^T [1200, 32] accumulate in ONE PSUM bank as [128p, 12 chunks, 32]
    (per-gate hd-chunks of 128/128/44 rows; gate order g,f,i,o; stationary
    weight columns are zero-padded to 128 so no partition garbage exists);
  - the input projection runs DIRECTLY into each step's PSUM bank
    (start=True) from resident xT tiles -- no gx buffer, no inject matmuls,
    no PSUM->SBUF drain;
  - h^T is produced by the elementwise chain already transposed -- the
    per-step PE transpose + copy of the h-major formulation disappears;
  - ONE sigmoid covers all 4 gates (host pre-doubles the g-gate rows;
    tanh(g) = 2*sigmoid(2g)-1 via a single 2x-mode tensor_scalar);
  - all elementwise ops are 2x-capable TensorTensor/TensorScalar at
    partition base 0 (no start-partition pairing issues);
  - ragged lengths: recurrence runs unmasked, Hf^T += h^T * mrep[:, :, t, :]
    with a one-hot-in-t replicated mask (two Pool ops, off critical path).
  - bias rides a ones-row in the xT k=2 tile (row 44) against a bias row in
    the Wih stationary; biases cost zero instructions.

This walrus build accepts only ONE sync wait per instruction, so after Tile
scheduling, extra waits are hoisted onto standalone EventSemaphore
instructions (_legalize_single_wait).
"""

import numpy as np
import ml_dtypes

import concourse.bass as bass
import concourse.tile as tile
from concourse import mybir
from concourse.bass_utils import run_bass_kernel_spmd

BF16 = ml_dtypes.bfloat16

# Model dims (hardcoded per spec nn_ABSA_Lstm_377957122440)
VOCAB, TVOCAB, D, H, C, B, T = 100000, 2000, 300, 300, 3, 256, 128
NCORES = 8
BL = B // NCORES          # 32 local batch
KT = 3                    # k-tiles over D/H: rows (128, 128, 44[+1 bias])
KW = (128, 128, 44)       # valid contraction rows per k-tile (H side)
KWX = (128, 128, 45)      # x side: k=2 carries the bias ones-row at row 44
NMC = 12                  # 4 gates x 3 hd-chunks of (128, 128, 44) rows
LAG = 1                   # p1-direct lookahead (steps)

_cache = {}


def _build_graph(legalize=True, debug=False, t_steps=T, reps=1,
                 trace_sim=False, variant=4):
    nc = bass.Bass()
    f32 = mybir.dt.float32
    bf16 = mybir.dt.bfloat16

    # ---- DRAM I/O (everything feature-major / transposed) ----
    xT = nc.dram_tensor("xT", [128, KT, T, BL], bf16, kind="ExternalInput")
    wihS = nc.dram_tensor("wihS", [128, KT, NMC, 128], bf16, kind="ExternalInput")
    whhS = nc.dram_tensor("whhS", [128, KT, NMC, 128], bf16, kind="ExternalInput")
    woutS = nc.dram_tensor("woutS", [128, KT, 8], bf16, kind="ExternalInput")
    boutT = nc.dram_tensor("boutT", [8, BL], f32, kind="ExternalInput")
    h0T = nc.dram_tensor("h0T", [128, KT, BL], bf16, kind="ExternalInput")
    c0T = nc.dram_tensor("c0T", [128, KT, BL], bf16, kind="ExternalInput")
    mrep = nc.dram_tensor("mrep", [128, KT, T, BL], bf16, kind="ExternalInput")
    outT = nc.dram_tensor("outT", [C, BL], f32, kind="ExternalOutput")
    dbg = None
    if debug:
        dbg = {
            "dbg_sig": nc.dram_tensor("dbg_sig", [128, NMC, BL], bf16,
                                      kind="ExternalOutput"),
            "dbg_h": nc.dram_tensor("dbg_h", [128, KT, BL], bf16,
                                    kind="ExternalOutput"),
        }

    with tile.TileContext(nc, trace_sim=trace_sim) as tc:
        for _ in range(reps):
            _body(nc, tc, xT, wihS, whhS, woutS, boutT, h0T, c0T, mrep,
                  outT, dbg, t_steps, variant)
    if legalize:
        _legalize_single_wait(nc)
    return nc


def _legalize_single_wait(nc):
    """This walrus build accepts at most ONE sync wait per instruction.
    Hoist extra waits emitted by Tile onto standalone EventSemaphore
    instructions placed immediately before the offender on the same engine."""
    for fn in nc.m.functions:
        for b in fn.blocks:
            out = []
            for inst in b.instructions:
                si = getattr(inst, "sync_info", None)
                if si is not None and si.on_wait and len(si.on_wait) > 1:
                    for w in si.on_wait[:-1]:
                        out.append(mybir.InstEventSemaphore(
                            name=nc.get_next_instruction_name(),
                            engine=inst.engine,
                            ins=[], outs=[],
                            sync_info=mybir.SyncInfo(on_wait=[w], on_update=[]),
                        ))
                    si.on_wait = [si.on_wait[-1]]
                out.append(inst)
            b.instructions[:] = out


def _body(nc, tc, xT, wihS, whhS, woutS, boutT, h0T, c0T, mrep, outT,
          dbg=None, t_steps=T, variant=4):
    f32 = mybir.dt.float32
    bf16 = mybir.dt.bfloat16
    Sig = mybir.ActivationFunctionType.Sigmoid
    Tanh = mybir.ActivationFunctionType.Tanh
    MUL = mybir.AluOpType.mult
    ADD = mybir.AluOpType.add

    from contextlib import ExitStack

    with ExitStack() as ctx:
        singles = ctx.enter_context(tc.tile_pool(name="singles", bufs=1))

        # ---- resident tiles (wih first: p1d(0) gates the loop start; xT
        # quartered so step 0 only waits 1/4 of its DMA; mrep on another
        # queue, only the off-path capture reads it) ----
        wih_sb = singles.tile([128, KT, NMC, 128], bf16)
        nc.sync.dma_start(out=wih_sb, in_=wihS[:, :, :, :])
        NQ, TQ = 4, T // 4
        xT_q = [singles.tile([128, KT, TQ, BL], bf16, name=f"xq{q}",
                             tag=f"xq{q}") for q in range(NQ)]
        for q in range(NQ):
            nc.sync.dma_start(out=xT_q[q],
                              in_=xT[:, :, q * TQ:(q + 1) * TQ, :])
        whh_sb = singles.tile([128, KT, NMC, 128], bf16)
        nc.sync.dma_start(out=whh_sb, in_=whhS[:, :, :, :])
        mrep_sb = singles.tile([128, KT, T, BL], bf16)
        nc.gpsimd.dma_start(out=mrep_sb, in_=mrep[:, :, :, :])

        HB = BL // 2          # batch halves: two independent pipelines
        cT = [[singles.tile([128, KT, HB], bf16, name=f"cT{h}_{i}",
                            tag=f"cT{h}_{i}") for i in range(2)]
              for h in range(2)]
        HfT = singles.tile([128, KT, BL], bf16)
        nc.vector.memset(HfT, 0.0)
        hT = [[singles.tile([128, KT, HB], bf16, name=f"hT{h}_{i}",
                            tag=f"hT{h}_{i}") for i in range(2)]
              for h in range(2)]
        for h in range(2):
            bs = slice(h * HB, (h + 1) * HB)
            nc.gpsimd.dma_start(out=cT[h][0], in_=c0T[:, :, bs])
            nc.gpsimd.dma_start(out=hT[h][0], in_=h0T[:, :, bs])
            nc.vector.memset(hT[h][1], 0.0)

        with ExitStack() as p2:
            pgpool = [p2.enter_context(
                tc.tile_pool(name=f"pgp{h}", bufs=3, space="PSUM"))
                for h in range(2)]
            ew = p2.enter_context(tc.tile_pool(name="ew", bufs=2))

            pg_tiles = {}

            def p1_direct(t):
                # input projection straight into step t's PSUM banks.
                # Gate chunks: f 0-2, g 3-5, i 6-8 in pg; o 0-2 in po (its
                # own bank so sigma(f,g,i) doesn't wait on o's matmuls:
                # dependencies are tile-granular).
                # start=True ONLY on the first matmul touching each bank:
                # the PSUM pending-zero region is the whole 2KB bank, so
                # each byte zeroes on its first write and accumulates after.
                pgs = [pgpool[h].tile([128, NMC, HB], f32,
                                      name=f"pg{t}_{h}", tag=f"pg{h}")
                       for h in range(2)]
                pg_tiles[t] = pgs
                xq = xT_q[t // TQ]
                for h in range(2):
                    bs = slice(h * HB, (h + 1) * HB)
                    for mc in range(NMC):
                        for k in range(KT):
                            nc.tensor.matmul(
                                pgs[h][:, mc, :],
                                lhsT=wih_sb[0:KWX[k], k, mc, :],
                                rhs=xq[0:KWX[k], k, t % TQ, bs],
                                start=(k == 0 and mc == 0), stop=False,
                                skip_group_check=True,
                            )

            for t in range(min(LAG + 1, t_steps)):
                p1_direct(t)

            for t in range(t_steps):
                pgs = pg_tiles.pop(t)

                for h in range(2):
                    cur = hT[h][t % 2]
                    for mc in range(NMC):
                        for k in range(KT):
                            nc.tensor.matmul(
                                pgs[h][:, mc, :],
                                lhsT=whh_sb[0:KW[k], k, mc, :],
                                rhs=cur[0:KW[k], k, :],
                                start=False,
                                stop=(k == KT - 1 and mc == NMC - 1),
                                skip_group_check=True,
                            )
                # prefetch next step's input projection (keeps PE warm
                # through the elementwise chain)
                if t + LAG + 1 < t_steps:
                    p1_direct(t + LAG + 1)

                if variant < 2:
                    continue

                # per batch-half pipelines; one sigma covers all gates
                sigTs = []
                for h in range(2):
                    sigT = ew.tile([128, NMC, HB], bf16)
                    nc.scalar.activation(sigT, pgs[h][:, :, :], Sig)
                    sigTs.append(sigT)
                if dbg is not None and t == 0:
                    for h in range(2):
                        bs = slice(h * HB, (h + 1) * HB)
                        nc.sync.dma_start(out=dbg["dbg_sig"][:, :, bs],
                                          in_=sigTs[h])

                if variant < 3:
                    continue
                # c-chain per half on DVE (program order, no internal hops):
                # fc = f*c; ig2 = (sig(2g)-0.5)*i = i*tanh(g)/2; c = 2*ig2+fc
                for h in range(2):
                    sigT = sigTs[h]
                    cold, cnew = cT[h][t % 2], cT[h][(t + 1) % 2]
                    fcT = ew.tile([128, KT, HB], bf16)
                    nc.vector.tensor_mul(fcT, sigT[:, 0:3, :], cold)
                    igT = ew.tile([128, KT, HB], bf16)
                    nc.vector.scalar_tensor_tensor(
                        igT, sigT[:, 3:6, :], 0.5, sigT[:, 6:9, :],
                        mybir.AluOpType.subtract, MUL)
                    nc.vector.scalar_tensor_tensor(
                        cnew, igT, 2.0, fcT, MUL, ADD)

                    # h = o * tanh(c) -> next h^T stationary directly
                    thT = ew.tile([128, KT, HB], bf16)
                    nc.scalar.activation(thT, cnew, Tanh)
                    nc.vector.tensor_mul(hT[h][(t + 1) % 2],
                                         sigT[:, 9:12, :], thT)

                if dbg is not None and t == 0:
                    for h in range(2):
                        bs = slice(h * HB, (h + 1) * HB)
                        nc.sync.dma_start(out=dbg["dbg_h"][:, :, bs],
                                          in_=hT[h][(t + 1) % 2])

                if variant < 4:
                    continue
                # ragged capture: Hf^T += h^T * mlast-broadcast (one-hot in t)
                for h in range(2):
                    bs = slice(h * HB, (h + 1) * HB)
                    hmT = ew.tile([128, KT, HB], bf16)
                    nc.gpsimd.tensor_mul(hmT, hT[h][(t + 1) % 2],
                                         mrep_sb[:, :, t, bs])
                    nc.gpsimd.tensor_add(HfT[:, :, bs], HfT[:, :, bs], hmT)

        # ================= phase 3: out^T = WoutT^T-chunks @ Hf^T ==========
        with ExitStack() as p3:
            fp = p3.enter_context(tc.tile_pool(name="fp", bufs=1))
            fps = p3.enter_context(tc.tile_pool(name="fps", bufs=1, space="PSUM"))
            wout_sb = fp.tile([128, KT, 8], bf16)
            nc.sync.dma_start(out=wout_sb, in_=woutS[:, :, :])
            bout_sb = fp.tile([8, BL], f32)
            nc.sync.dma_start(out=bout_sb, in_=boutT[:, :])
            po = fps.tile([8, BL], f32)
            for k in range(KT):
                nc.tensor.matmul(po[0:C, :], lhsT=wout_sb[0:KW[k], k, 0:C],
                                 rhs=HfT[0:KW[k], k, :],
                                 start=(k == 0), stop=(k == KT - 1))
            o_sb = fp.tile([C, BL], f32)
            nc.vector.tensor_add(o_sb, po[0:C, :], bout_sb[0:C, :])
            nc.sync.dma_start(out=outT[:, :], in_=o_sb)


def _prep_inputs(sent, target, lens, emb, Wih, Whh, b_ih, b_hh, h0, c0,
                 Wout, bout):
    """Host-side shard + layout packing (data movement / tiny reindexing only)."""
    # permute gate order (i,f,g,o) -> (f,g,i,o); double g rows for the
    # tanh(g) = 2*sigmoid(2g) - 1 trick.
    perm = np.concatenate([np.arange(300, 600), np.arange(600, 900),
                           np.arange(0, 300), np.arange(900, 1200)])
    wih_r = Wih[perm].astype(np.float32)          # [1200, 300]
    whh_r = Whh[perm].astype(np.float32)
    bias_r = (b_ih + b_hh)[perm].astype(np.float32)
    wih_r[300:600] *= 2.0
    whh_r[300:600] *= 2.0
    bias_r[300:600] *= 2.0

    # stationaries: [k-rows, k, mc, m-cols], zero-padded m-cols
    wihS = np.zeros((128, KT, NMC, 128), np.float32)
    whhS = np.zeros((128, KT, NMC, 128), np.float32)
    for mc in range(NMC):
        g = mc // 3
        lo = 300 * g + 128 * (mc % 3)
        hi = min(300 * (g + 1), lo + 128)
        for k in range(KT):
            klo, khi = 128 * k, min(128 * (k + 1), D)
            wihS[0:khi - klo, k, mc, 0:hi - lo] = wih_r[lo:hi, klo:khi].T
            whhS[0:khi - klo, k, mc, 0:hi - lo] = whh_r[lo:hi, klo:khi].T
        wihS[44, 2, mc, 0:hi - lo] = bias_r[lo:hi]     # ones-row partner
    woutS = np.zeros((128, KT, 8), np.float32)
    for k in range(KT):
        klo, khi = 128 * k, min(128 * (k + 1), H)
        woutS[0:khi - klo, k, 0:C] = Wout[:, klo:khi].T

    in_maps = []
    for ci in range(NCORES):
        sl = slice(ci * BL, (ci + 1) * BL)
        x = emb[sent[sl]].astype(np.float32)       # [32, 128, 300] gather
        xT = np.zeros((128, KT, T, BL), np.float32)
        for k in range(KT):
            klo, khi = 128 * k, min(128 * (k + 1), D)
            xT[0:khi - klo, k, :, :] = x[:, :, klo:khi].transpose(2, 1, 0)
        xT[44, 2, :, :] = 1.0                      # bias ones-row

        h0T = np.zeros((128, KT, BL), np.float32)
        c0T = np.zeros((128, KT, BL), np.float32)
        for k in range(KT):
            klo, khi = 128 * k, min(128 * (k + 1), H)
            h0T[0:khi - klo, k, :] = h0[sl, klo:khi].T
            c0T[0:khi - klo, k, :] = c0[sl, klo:khi].T

        lloc = lens[sl].astype(np.int64)
        mlast = np.zeros((BL, T), np.float32)
        mlast[np.arange(BL), np.clip(lloc - 1, 0, T - 1)] = 1.0
        mrep = np.broadcast_to(mlast.T[None, None, :, :],
                               (128, KT, T, BL)).copy()

        in_maps.append({
            "xT": xT.astype(BF16),
            "wihS": wihS.astype(BF16),
            "whhS": whhS.astype(BF16),
            "woutS": woutS.astype(BF16),
            "boutT": np.pad(np.tile(bout.astype(np.float32)[:, None],
                                    (1, BL)), ((0, 8 - C), (0, 0))),
            "h0T": h0T.astype(BF16),
            "c0T": c0T.astype(BF16),
            "mrep": mrep.astype(BF16),
        })
    return in_maps


def kernel(**inputs):
    if "nc" not in _cache:
        _cache["nc"] = _build_graph()
    nc = _cache["nc"]
    in_maps = _prep_inputs(**inputs)
    res = run_bass_kernel_spmd(nc, in_maps, core_ids=list(range(NCORES)))
    outs = [res.results[i]["outT"].T for i in range(NCORES)]
    return np.concatenate(outs, axis=0).astype(np.float32)


# revision 21
# speedup vs baseline: 2.9437x; 1.0443x over previous
"""
Trainium2 Bass kernel for nn_ABSA_Lstm: masked LSTM over ragged sequences.

  reference:  x = emb[sent]; LSTM over T=128 steps with per-sequence length
              masking; out = h_final @ Wout.T + bout   -> [256, 3]

Strategy (8 NeuronCores, data parallel):
  - Shard batch B=256 -> 32 sequences per core. LSTM weights replicated.
  - Host does the embedding-table gather (pure data movement) and packs
    transposed/padded tile layouts; all model FLOPs run on device.

FULLY TRANSPOSED formulation: every on-chip tensor carries feature dims on
PARTITIONS and batch on the FREE dim.  Engine cost on TRN2 is free-size
driven, so with batch (32) as the free dim every instruction is near its
fixed-cost floor, and:
  - gates^T [1200, 32] accumulate in ONE PSUM bank as [128p, 12 chunks, 32]
    (per-gate hd-chunks of 128/128/44 rows; gate order g,f,i,o; stationary
    weight columns are zero-padded to 128 so no partition garbage exists);
  - the input projection runs DIRECTLY into each step's PSUM bank
    (start=True) from resident xT tiles -- no gx buffer, no inject matmuls,
    no PSUM->SBUF drain;
  - h^T is produced by the elementwise chain already transposed -- the
    per-step PE transpose + copy of the h-major formulation disappears;
  - ONE sigmoid covers all 4 gates (host pre-doubles the g-gate rows;
    tanh(g) = 2*sigmoid(2g)-1 via a single 2x-mode tensor_scalar);
  - all elementwise ops are 2x-capable TensorTensor/TensorScalar at
    partition base 0 (no start-partition pairing issues);
  - ragged lengths: recurrence runs unmasked, Hf^T += h^T * mrep[:, :, t, :]
    with a one-hot-in-t replicated mask (two Pool ops, off critical path).
  - bias rides a ones-row in the xT k=2 tile (row 44) against a bias row in
    the Wih stationary; biases cost zero instructions.

This walrus build accepts only ONE sync wait per instruction, so after Tile
scheduling, extra waits are hoisted onto standalone EventSemaphore
instructions (_legalize_single_wait).
"""

import numpy as np
import ml_dtypes

import concourse.bass as bass
import concourse.tile as tile
from concourse import mybir
from concourse.bass_utils import run_bass_kernel_spmd

BF16 = ml_dtypes.bfloat16

# Model dims (hardcoded per spec nn_ABSA_Lstm_377957122440)
VOCAB, TVOCAB, D, H, C, B, T = 100000, 2000, 300, 300, 3, 256, 128
NCORES = 8
BL = B // NCORES          # 32 local batch
KT = 3                    # k-tiles over D/H: rows (128, 128, 44[+1 bias])
KW = (128, 128, 44)       # valid contraction rows per k-tile (H side)
KWX = (128, 128, 45)      # x side: k=2 carries the bias ones-row at row 44
NMC = 12                  # 4 gates x 3 hd-chunks of (128, 128, 44) rows
LAG = 1                   # p1-direct lookahead (steps)

_cache = {}


def _build_graph(legalize=True, debug=False, t_steps=T, reps=1,
                 trace_sim=False, variant=4):
    nc = bass.Bass()
    f32 = mybir.dt.float32
    bf16 = mybir.dt.bfloat16

    # ---- DRAM I/O (everything feature-major / transposed) ----
    xT = nc.dram_tensor("xT", [128, KT, T, BL], bf16, kind="ExternalInput")
    wihS = nc.dram_tensor("wihS", [128, KT, NMC, 128], bf16, kind="ExternalInput")
    whhS = nc.dram_tensor("whhS", [128, KT, NMC, 128], bf16, kind="ExternalInput")
    woutS = nc.dram_tensor("woutS", [128, KT, 8], bf16, kind="ExternalInput")
    boutT = nc.dram_tensor("boutT", [8, BL], f32, kind="ExternalInput")
    h0T = nc.dram_tensor("h0T", [128, KT, BL], bf16, kind="ExternalInput")
    c0T = nc.dram_tensor("c0T", [128, KT, BL], bf16, kind="ExternalInput")
    mrep = nc.dram_tensor("mrep", [128, KT, T, BL], bf16, kind="ExternalInput")
    outT = nc.dram_tensor("outT", [C, BL], f32, kind="ExternalOutput")
    dbg = None
    if debug:
        dbg = {
            "dbg_sig": nc.dram_tensor("dbg_sig", [128, NMC, BL], bf16,
                                      kind="ExternalOutput"),
            "dbg_h": nc.dram_tensor("dbg_h", [128, KT, BL], bf16,
                                    kind="ExternalOutput"),
        }

    with tile.TileContext(nc, trace_sim=trace_sim) as tc:
        for _ in range(reps):
            _body(nc, tc, xT, wihS, whhS, woutS, boutT, h0T, c0T, mrep,
                  outT, dbg, t_steps, variant)
    if legalize:
        _legalize_single_wait(nc)
    return nc


def _legalize_single_wait(nc):
    """This walrus build accepts at most ONE sync wait per instruction.
    Hoist extra waits emitted by Tile onto standalone EventSemaphore
    instructions placed immediately before the offender on the same engine."""
    for fn in nc.m.functions:
        for b in fn.blocks:
            out = []
            for inst in b.instructions:
                si = getattr(inst, "sync_info", None)
                if si is not None and si.on_wait and len(si.on_wait) > 1:
                    for w in si.on_wait[:-1]:
                        out.append(mybir.InstEventSemaphore(
                            name=nc.get_next_instruction_name(),
                            engine=inst.engine,
                            ins=[], outs=[],
                            sync_info=mybir.SyncInfo(on_wait=[w], on_update=[]),
                        ))
                    si.on_wait = [si.on_wait[-1]]
                out.append(inst)
            b.instructions[:] = out


def _body(nc, tc, xT, wihS, whhS, woutS, boutT, h0T, c0T, mrep, outT,
          dbg=None, t_steps=T, variant=4):
    f32 = mybir.dt.float32
    bf16 = mybir.dt.bfloat16
    Sig = mybir.ActivationFunctionType.Sigmoid
    Tanh = mybir.ActivationFunctionType.Tanh
    MUL = mybir.AluOpType.mult
    ADD = mybir.AluOpType.add

    from contextlib import ExitStack

    with ExitStack() as ctx:
        singles = ctx.enter_context(tc.tile_pool(name="singles", bufs=1))

        # ---- resident tiles (wih first: p1d(0) gates the loop start; xT
        # quartered so step 0 only waits 1/4 of its DMA; mrep on another
        # queue, only the off-path capture reads it) ----
        wih_sb = singles.tile([128, KT, NMC, 128], bf16)
        nc.sync.dma_start(out=wih_sb, in_=wihS[:, :, :, :])
        NQ, TQ = 4, T // 4
        xT_q = [singles.tile([128, KT, TQ, BL], bf16, name=f"xq{q}",
                             tag=f"xq{q}") for q in range(NQ)]
        for q in range(NQ):
            nc.sync.dma_start(out=xT_q[q],
                              in_=xT[:, :, q * TQ:(q + 1) * TQ, :])
        whh_sb = singles.tile([128, KT, NMC, 128], bf16)
        nc.sync.dma_start(out=whh_sb, in_=whhS[:, :, :, :])
        mrep_sb = singles.tile([128, KT, T, BL], bf16)
        nc.gpsimd.dma_start(out=mrep_sb, in_=mrep[:, :, :, :])

        HB = BL // 2          # batch halves: two independent pipelines
        cT = [[singles.tile([128, KT, HB], bf16, name=f"cT{h}_{i}",
                            tag=f"cT{h}_{i}") for i in range(2)]
              for h in range(2)]
        HfT = singles.tile([128, KT, BL], bf16)
        nc.vector.memset(HfT, 0.0)
        hT = [[singles.tile([128, KT, HB], bf16, name=f"hT{h}_{i}",
                            tag=f"hT{h}_{i}") for i in range(2)]
              for h in range(2)]
        for h in range(2):
            bs = slice(h * HB, (h + 1) * HB)
            nc.gpsimd.dma_start(out=cT[h][0], in_=c0T[:, :, bs])
            nc.gpsimd.dma_start(out=hT[h][0], in_=h0T[:, :, bs])
            nc.vector.memset(hT[h][1], 0.0)

        with ExitStack() as p2:
            pgpool = [p2.enter_context(
                tc.tile_pool(name=f"pgp{h}", bufs=3, space="PSUM"))
                for h in range(2)]
            ew = p2.enter_context(tc.tile_pool(name="ew", bufs=2))

            pg_tiles = {}

            def p1_direct(t):
                # input projection straight into step t's PSUM banks.
                # Gate chunks: f 0-2, g 3-5, i 6-8 in pg; o 0-2 in po (its
                # own bank so sigma(f,g,i) doesn't wait on o's matmuls:
                # dependencies are tile-granular).
                # start=True ONLY on the first matmul touching each bank:
                # the PSUM pending-zero region is the whole 2KB bank, so
                # each byte zeroes on its first write and accumulates after.
                pgs = [pgpool[h].tile([128, NMC, HB], f32,
                                      name=f"pg{t}_{h}", tag=f"pg{h}")
                       for h in range(2)]
                pg_tiles[t] = pgs
                xq = xT_q[t // TQ]
                for h in range(2):
                    bs = slice(h * HB, (h + 1) * HB)
                    for mc in range(NMC):
                        for k in range(KT):
                            nc.tensor.matmul(
                                pgs[h][:, mc, :],
                                lhsT=wih_sb[0:KWX[k], k, mc, :],
                                rhs=xq[0:KWX[k], k, t % TQ, bs],
                                start=(k == 0 and mc == 0), stop=False,
                                skip_group_check=True,
                            )

            for t in range(min(LAG + 1, t_steps)):
                p1_direct(t)

            for t in range(t_steps):
                pgs = pg_tiles.pop(t)

                for h in range(2):
                    cur = hT[h][t % 2]
                    for mc in range(NMC):
                        for k in range(KT):
                            nc.tensor.matmul(
                                pgs[h][:, mc, :],
                                lhsT=whh_sb[0:KW[k], k, mc, :],
                                rhs=cur[0:KW[k], k, :],
                                start=False,
                                stop=(k == KT - 1 and mc == NMC - 1),
                                skip_group_check=True,
                            )
                # prefetch next step's input projection (keeps PE warm
                # through the elementwise chain)
                if t + LAG + 1 < t_steps:
                    p1_direct(t + LAG + 1)

                if variant < 2:
                    continue

                # per batch-half pipelines; one sigma covers all gates
                sigTs = []
                for h in range(2):
                    sigT = ew.tile([128, NMC, HB], bf16)
                    nc.scalar.activation(sigT, pgs[h][:, :, :], Sig)
                    sigTs.append(sigT)
                if dbg is not None and t == 0:
                    for h in range(2):
                        bs = slice(h * HB, (h + 1) * HB)
                        nc.sync.dma_start(out=dbg["dbg_sig"][:, :, bs],
                                          in_=sigTs[h])

                if variant < 3:
                    continue
                # c-chain per half on DVE (program order, no internal hops):
                # fc = f*c; ig2 = (sig(2g)-0.5)*i = i*tanh(g)/2; c = 2*ig2+fc
                for h in range(2):
                    sigT = sigTs[h]
                    cold, cnew = cT[h][t % 2], cT[h][(t + 1) % 2]
                    igT = ew.tile([128, KT, HB], bf16)
                    nc.vector.scalar_tensor_tensor(
                        igT, sigT[:, 3:6, :], 0.5, sigT[:, 6:9, :],
                        mybir.AluOpType.subtract, MUL)
                    fcT = ew.tile([128, KT, HB], bf16)
                    nc.gpsimd.tensor_mul(fcT, sigT[:, 0:3, :], cold)
                    nc.vector.scalar_tensor_tensor(
                        cnew, igT, 2.0, fcT, MUL, ADD)

                    # h = o * tanh(c) -> next h^T stationary directly
                    thT = ew.tile([128, KT, HB], bf16)
                    nc.scalar.activation(thT, cnew, Tanh)
                    nc.vector.tensor_mul(hT[h][(t + 1) % 2],
                                         sigT[:, 9:12, :], thT)

                if dbg is not None and t == 0:
                    for h in range(2):
                        bs = slice(h * HB, (h + 1) * HB)
                        nc.sync.dma_start(out=dbg["dbg_h"][:, :, bs],
                                          in_=hT[h][(t + 1) % 2])

                if variant < 4:
                    continue
                # ragged capture: Hf^T += h^T * mlast-broadcast (one-hot in t)
                for h in range(2):
                    bs = slice(h * HB, (h + 1) * HB)
                    hmT = ew.tile([128, KT, HB], bf16)
                    nc.gpsimd.tensor_mul(hmT, hT[h][(t + 1) % 2],
                                         mrep_sb[:, :, t, bs])
                    nc.gpsimd.tensor_add(HfT[:, :, bs], HfT[:, :, bs], hmT)

        # ================= phase 3: out^T = WoutT^T-chunks @ Hf^T ==========
        with ExitStack() as p3:
            fp = p3.enter_context(tc.tile_pool(name="fp", bufs=1))
            fps = p3.enter_context(tc.tile_pool(name="fps", bufs=1, space="PSUM"))
            wout_sb = fp.tile([128, KT, 8], bf16)
            nc.sync.dma_start(out=wout_sb, in_=woutS[:, :, :])
            bout_sb = fp.tile([8, BL], f32)
            nc.sync.dma_start(out=bout_sb, in_=boutT[:, :])
            po = fps.tile([8, BL], f32)
            for k in range(KT):
                nc.tensor.matmul(po[0:C, :], lhsT=wout_sb[0:KW[k], k, 0:C],
                                 rhs=HfT[0:KW[k], k, :],
                                 start=(k == 0), stop=(k == KT - 1))
            o_sb = fp.tile([C, BL], f32)
            nc.vector.tensor_add(o_sb, po[0:C, :], bout_sb[0:C, :])
            nc.sync.dma_start(out=outT[:, :], in_=o_sb)


def _prep_inputs(sent, target, lens, emb, Wih, Whh, b_ih, b_hh, h0, c0,
                 Wout, bout):
    """Host-side shard + layout packing (data movement / tiny reindexing only)."""
    # permute gate order (i,f,g,o) -> (f,g,i,o); double g rows for the
    # tanh(g) = 2*sigmoid(2g) - 1 trick.
    perm = np.concatenate([np.arange(300, 600), np.arange(600, 900),
                           np.arange(0, 300), np.arange(900, 1200)])
    wih_r = Wih[perm].astype(np.float32)          # [1200, 300]
    whh_r = Whh[perm].astype(np.float32)
    bias_r = (b_ih + b_hh)[perm].astype(np.float32)
    wih_r[300:600] *= 2.0
    whh_r[300:600] *= 2.0
    bias_r[300:600] *= 2.0

    # stationaries: [k-rows, k, mc, m-cols], zero-padded m-cols
    wihS = np.zeros((128, KT, NMC, 128), np.float32)
    whhS = np.zeros((128, KT, NMC, 128), np.float32)
    for mc in range(NMC):
        g = mc // 3
        lo = 300 * g + 128 * (mc % 3)
        hi = min(300 * (g + 1), lo + 128)
        for k in range(KT):
            klo, khi = 128 * k, min(128 * (k + 1), D)
            wihS[0:khi - klo, k, mc, 0:hi - lo] = wih_r[lo:hi, klo:khi].T
            whhS[0:khi - klo, k, mc, 0:hi - lo] = whh_r[lo:hi, klo:khi].T
        wihS[44, 2, mc, 0:hi - lo] = bias_r[lo:hi]     # ones-row partner
    woutS = np.zeros((128, KT, 8), np.float32)
    for k in range(KT):
        klo, khi = 128 * k, min(128 * (k + 1), H)
        woutS[0:khi - klo, k, 0:C] = Wout[:, klo:khi].T

    in_maps = []
    for ci in range(NCORES):
        sl = slice(ci * BL, (ci + 1) * BL)
        x = emb[sent[sl]].astype(np.float32)       # [32, 128, 300] gather
        xT = np.zeros((128, KT, T, BL), np.float32)
        for k in range(KT):
            klo, khi = 128 * k, min(128 * (k + 1), D)
            xT[0:khi - klo, k, :, :] = x[:, :, klo:khi].transpose(2, 1, 0)
        xT[44, 2, :, :] = 1.0                      # bias ones-row

        h0T = np.zeros((128, KT, BL), np.float32)
        c0T = np.zeros((128, KT, BL), np.float32)
        for k in range(KT):
            klo, khi = 128 * k, min(128 * (k + 1), H)
            h0T[0:khi - klo, k, :] = h0[sl, klo:khi].T
            c0T[0:khi - klo, k, :] = c0[sl, klo:khi].T

        lloc = lens[sl].astype(np.int64)
        mlast = np.zeros((BL, T), np.float32)
        mlast[np.arange(BL), np.clip(lloc - 1, 0, T - 1)] = 1.0
        mrep = np.broadcast_to(mlast.T[None, None, :, :],
                               (128, KT, T, BL)).copy()

        in_maps.append({
            "xT": xT.astype(BF16),
            "wihS": wihS.astype(BF16),
            "whhS": whhS.astype(BF16),
            "woutS": woutS.astype(BF16),
            "boutT": np.pad(np.tile(bout.astype(np.float32)[:, None],
                                    (1, BL)), ((0, 8 - C), (0, 0))),
            "h0T": h0T.astype(BF16),
            "c0T": c0T.astype(BF16),
            "mrep": mrep.astype(BF16),
        })
    return in_maps


def kernel(**inputs):
    if "nc" not in _cache:
        _cache["nc"] = _build_graph()
    nc = _cache["nc"]
    in_maps = _prep_inputs(**inputs)
    res = run_bass_kernel_spmd(nc, in_maps, core_ids=list(range(NCORES)))
    outs = [res.results[i]["outT"].T for i in range(NCORES)]
    return np.concatenate(outs, axis=0).astype(np.float32)
